# revision 19
# baseline (speedup 1.0000x reference)
"""Trainium2 Bass kernel for the BMN-style nn module (nn_BMN_66683662238004).

Pipeline (per batch b):
  base = relu(conv1d(relu(conv1d(x))))            # [128, T]
  tem_out = sigmoid(conv1d(relu(conv1d(base))))   # [2, T]
  pem[c, n, d, t] = sum_tt base[c, tt] * Wsmp[tt, n, d, t]   (BM sampling)
  y1 = relu(conv3d(pem))   == per-(d,t) column: sum over n of W3_n[c,o] @ pem[c,n,(d,t)]
  z1 = relu(1x1(y1)); z2 = relu(3x3(z1)); y = sigmoid(1x1(z2))

Sharding: 8 cores, each owns a contiguous window of 13 t-columns (plus 1-col
halo each side for the 3x3 conv). Wsmp is precomputed on host (it is a
constant sparse interpolation matrix) and shipped pre-sliced per core.
All heavy matmuls run in fp32r (TF32-like) on the PE array.
"""

import os
import sys
import threading

import numpy as np

# ---------------------------------------------------------------- constants
T, NSMP, DPROP, EXPAND = 100, 32, 100, 0.5
FEAT, BATCH = 400, 4
NCORES = 8
TC = 13           # output t-columns per core (8*13 = 104 >= 100)
TW = TC + 2       # t-window incl halo
NCOLS = TW * 100  # packed phase-A columns per core (t-major, d-minor)
CB = 500          # phase-A column block (<=512 psum, >=256 for fp32r full rate)
NBLK = NCOLS // CB  # 3

_cache = {}


# ---------------------------------------------------------------- host math
def _smp_w4():
    """Faithful BMSampling weight, laid out [tt, n, t, d] float32."""
    ii = np.arange(T)                    # t (start index i)
    jj = np.arange(DPROP)                # d (duration index j)
    kk = np.arange(NSMP)
    J, I = np.meshgrid(jj, ii, indexing="ij")        # [d, t]
    valid_ij = J < np.minimum(T - 1 - I, DPROP)      # j < min(T-1-i, D); i<=T-2 implied
    length = (J + 1 - I).astype(np.float64)
    xmin_ext = I - length * EXPAND
    bin_size = (length + 2 * EXPAND * length) / (NSMP - 1)
    xp = xmin_ext[None] + kk[:, None, None] * bin_size[None]   # [n, d, t]
    ok = valid_ij[None] & (xp >= 0) & (xp <= T - 1)
    left = np.floor(xp).astype(np.int64)
    right = np.ceil(xp).astype(np.int64)
    wl = 1.0 - (xp - left)
    wr = 1.0 - (right - xp)
    w = np.zeros((T, NSMP, T, DPROP), np.float32)    # [tt, n, t, d]
    n_i, d_i, t_i = np.nonzero(ok)
    np.add.at(w, (left[ok], n_i, t_i, d_i), wl[ok])
    np.add.at(w, (right[ok], n_i, t_i, d_i), wr[ok])
    return w


def _prep_host(inputs):
    """Host-side constant prep: Wsmp slices + transposed weights."""
    import ml_dtypes
    bf16 = ml_dtypes.bfloat16
    w4 = _smp_w4()                                   # [tt, n, t, d]
    wsmps, tvals = [], []
    for r in range(NCORES):
        t0 = r * TC - 1
        sl = np.zeros((T, NSMP, TW, DPROP), np.float32)
        lo, hi = max(0, t0), min(T, t0 + TW)
        sl[:, :, lo - t0 : hi - t0, :] = w4[:, :, lo:hi, :]
        wsmps.append(np.ascontiguousarray(sl.reshape(T, NSMP, NCOLS).astype(bf16)))
        tv = np.zeros(TW, np.float32)
        tv[lo - t0 : hi - t0] = 1.0
        tvals.append(tv)

    pr = {
        "wb1t": np.ascontiguousarray(inputs["w_base1"].transpose(1, 0, 2)),  # [400,256,3]
        "wb2t": np.ascontiguousarray(inputs["w_base2"].transpose(1, 0, 2)),  # [256,128,3]
        "wt1t": np.ascontiguousarray(inputs["w_tem1"].transpose(1, 0, 2)),   # [128,256,3]
        "wt2t": np.ascontiguousarray(inputs["w_tem2"].transpose(1, 0, 2)),   # [256,2,3]
        "w3r": np.ascontiguousarray(inputs["w_c3d"].transpose(2, 1, 0)),     # [32,128,512]
        "w1x1t": np.ascontiguousarray(
            inputs["w_2d1"].reshape(128, 512).transpose(1, 0)),              # [512,128]
        "w2d2t": np.ascontiguousarray(
            inputs["w_2d2"].transpose(2, 3, 1, 0).reshape(9, 128, 128)),     # [kh*3+kw,c,o]
        "w2d3t": np.ascontiguousarray(inputs["w_2d3"].reshape(2, 128).transpose(1, 0)),
    }
    for k in ("w3r", "w1x1t", "w2d2t"):
        pr[k] = pr[k].astype(bf16)
    return wsmps, tvals, pr


# ---------------------------------------------------------------- device build
def _build_program():
    import concourse.bass as bass
    import concourse.tile as tile
    from concourse import bacc, mybir
    from concourse.masks import make_identity

    f32 = mybir.dt.float32
    f32r = mybir.dt.float32r
    bf16 = mybir.dt.bfloat16
    AF = mybir.ActivationFunctionType

    nc = bacc.Bacc("TRN2", target_bir_lowering=False, debug=False,
                   num_devices=NCORES)

    def din(name, shape):
        return nc.dram_tensor(name, shape, f32, kind="ExternalInput").ap()

    x_d = din("x", [BATCH, FEAT, T])
    zeros_d = din("zeros", [512])
    zerosh_d = nc.dram_tensor("zerosh", [512], mybir.dt.bfloat16,
                              kind="ExternalInput").ap()
    wsmp_d = nc.dram_tensor("wsmp", [T, NSMP, NCOLS], mybir.dt.bfloat16,
                            kind="ExternalInput").ap()
    tval_d = din("tval", [TW])
    wb1t_d = din("wb1t", [400, 256, 3])
    b1_d = din("b_base1", [256])
    wb2t_d = din("wb2t", [256, 128, 3])
    b2_d = din("b_base2", [128])
    wt1t_d = din("wt1t", [128, 256, 3])
    bt1_d = din("b_tem1", [256])
    wt2t_d = din("wt2t", [256, 2, 3])
    bt2_d = din("b_tem2", [2])
    w3r_d = nc.dram_tensor("w3r", [NSMP, 128, 512], mybir.dt.bfloat16,
                           kind="ExternalInput").ap()
    b3_d = din("b_c3d", [512])
    w1x1t_d = nc.dram_tensor("w1x1t", [512, 128], mybir.dt.bfloat16,
                             kind="ExternalInput").ap()
    b2d1_d = din("b_2d1", [128])
    w2d2t_d = nc.dram_tensor("w2d2t", [9, 128, 128], mybir.dt.bfloat16,
                             kind="ExternalInput").ap()
    b2d2_d = din("b_2d2", [128])
    w2d3t_d = din("w2d3t", [128, 2])
    b2d3_d = din("b_2d3", [2])

    # stored (t, d) on device; host transposes to (d, t)
    y_d = nc.dram_tensor("y", [BATCH, 2, TC, DPROP], f32, kind="ExternalOutput").ap()
    tem_d = nc.dram_tensor("tem", [BATCH, 2, T], f32, kind="ExternalOutput").ap()

    with tile.TileContext(nc) as tc:
        # ---------------- persistent pools
        persist = tc.alloc_tile_pool(name="persist", bufs=1)
        z1_pool = tc.alloc_tile_pool(name="z1", bufs=1)

        ident = persist.tile([128, 128], f32)
        make_identity(nc, ident)

        # biases as [p,1] tiles
        def bias_tiles(src, n_chunks, tag):
            ts = []
            for i in range(n_chunks):
                t_ = persist.tile([128, 1], f32, tag=f"{tag}{i}")
                nc.sync.dma_start(out=t_[:, 0], in_=src[i * 128:(i + 1) * 128])
                ts.append(t_)
            return ts

        b1sb = bias_tiles(b1_d, 2, "b1")
        b2sb = bias_tiles(b2_d, 1, "b2")
        bt1sb = bias_tiles(bt1_d, 2, "bt1")
        b3sb = bias_tiles(b3_d, 4, "b3")
        b2d1sb = bias_tiles(b2d1_d, 1, "b2d1")
        b2d2sb = bias_tiles(b2d2_d, 1, "b2d2")
        bt2sb = persist.tile([2, 1], f32)
        nc.sync.dma_start(out=bt2sb[:, 0], in_=bt2_d[:])
        b2d3sb = persist.tile([2, 1], f32)
        nc.sync.dma_start(out=b2d3sb[:, 0], in_=b2d3_d[:])

        tvalsb = persist.tile([128, TW], f32)
        nc.sync.dma_start(
            out=tvalsb[:],
            in_=bass.AP(tensor=tval_d.tensor, offset=tval_d.offset,
                        ap=[[0, 128], *tval_d.ap]))

        def dma_zero(out_ap):
            """Zero-fill an f32r/f32 tile region via DMA from the zeros input
            (memset ISA does not support float32r)."""
            dims = out_ap.shape
            if out_ap.dtype == bf16:
                srct = zerosh_d
            elif out_ap.dtype == f32:
                srct = zeros_d
            else:
                srct = zeros_d.bitcast(out_ap.dtype)
            ap = [[0, dims[0]]] + [[0, d] for d in dims[1:-1]] + [[1, dims[-1]]]
            nc.sync.dma_start(
                out=out_ap,
                in_=bass.AP(tensor=srct.tensor, offset=srct.offset, ap=ap))

        # w3r resident [32][128, 512], w1x1 [4][128,128], w2d2 taps, w2d3
        w3rsb = []
        for n in range(NSMP):
            w_ = persist.tile([128, 512], bf16, tag=f"w3r{n}")
            nc.sync.dma_start(out=w_[:], in_=w3r_d[n])
            w3rsb.append(w_)
        w1x1sb = []
        for oc in range(4):
            w_ = persist.tile([128, 128], bf16, tag=f"w1x1_{oc}")
            nc.sync.dma_start(out=w_[:], in_=w1x1t_d[oc * 128:(oc + 1) * 128, :])
            w1x1sb.append(w_)
        w2d2sb = []
        for tap in range(9):
            w_ = persist.tile([128, 128], bf16, tag=f"w2d2_{tap}")
            nc.sync.dma_start(out=w_[:], in_=w2d2t_d[tap])
            w2d2sb.append(w_)
        # wsmp fully resident in SBUF (bf16): one block per phase-A col block
        wsmpsb = persist.tile([100, NSMP, NCOLS], bf16)
        for blk in range(NBLK):
            nc.sync.dma_start(
                out=wsmpsb[:, :, blk * CB:(blk + 1) * CB],
                in_=wsmp_d[:, :, blk * CB:(blk + 1) * CB])
        w2d3sb = persist.tile([128, 2], f32)
        nc.sync.dma_start(out=w2d3sb[:], in_=w2d3t_d[:])

        # z1 maps: [128, TW trows, 102 dcols] per b  (d-pad cols 0 and 101)
        z1 = [z1_pool.tile([128, TW, 102], bf16, tag=f"z1b{b}", name=f"z1b{b}")
              for b in range(BATCH)]

        baseT = [persist.tile([100, 128], bf16, tag=f"baseT{b}", name=f"baseT{b}")
                 for b in range(BATCH)]

        # ---------------- front: conv1d stack + TEM + transposes
        with tc.tile_pool(name="front", bufs=1) as fr, \
             tc.tile_pool(name="front_ps", bufs=1, space="PSUM") as frps:
            # x -> sbuf [100, 4, 102] x4 chunks, t-padded
            x_sb = []
            for kc in range(4):
                t_ = fr.tile([100, BATCH, 102], f32r, tag=f"x{kc}")
                dma_zero(t_[:, :, 0:1])
                dma_zero(t_[:, :, 101:102])
                nc.sync.dma_start(
                    out=t_[:, :, 1:101],
                    in_=x_d[:, kc * 100:(kc + 1) * 100, :].rearrange("b c t -> c b t").bitcast(f32r))
                x_sb.append(t_)
            # conv weights
            wb1sb = {}
            for kc in range(4):
                for mc in range(2):
                    for tap in range(3):
                        w_ = fr.tile([100, 128], f32r, tag=f"wb1_{kc}_{mc}_{tap}")
                        nc.sync.dma_start(
                            out=w_[:],
                            in_=wb1t_d[kc * 100:(kc + 1) * 100,
                                       mc * 128:(mc + 1) * 128, tap].bitcast(f32r))
                        wb1sb[kc, mc, tap] = w_
            wb2sb = {}
            for kc in range(2):
                for tap in range(3):
                    w_ = fr.tile([128, 128], f32r, tag=f"wb2_{kc}_{tap}")
                    nc.sync.dma_start(
                        out=w_[:],
                        in_=wb2t_d[kc * 128:(kc + 1) * 128, :, tap].bitcast(f32r))
                    wb2sb[kc, tap] = w_
            wt1sb = {}
            for mc in range(2):
                for tap in range(3):
                    w_ = fr.tile([128, 128], f32, tag=f"wt1_{mc}_{tap}")
                    nc.sync.dma_start(
                        out=w_[:],
                        in_=wt1t_d[:, mc * 128:(mc + 1) * 128, tap])
                    wt1sb[mc, tap] = w_
            wt2sb = {}
            for kc in range(2):
                for tap in range(3):
                    w_ = fr.tile([128, 2], f32, tag=f"wt2_{kc}_{tap}")
                    nc.sync.dma_start(
                        out=w_[:],
                        in_=wt2t_d[kc * 128:(kc + 1) * 128, :, tap])
                    wt2sb[kc, tap] = w_

            # base1 = relu(conv1d(x))  [256 -> 2 chunks][100 t x 4 b]
            base1_sb = []
            for mc in range(2):
                ps = frps.tile([128, BATCH, 100], f32, tag="ps_b1")
                first = True
                for kc in range(4):
                    for tap in range(3):
                        nc.tensor.matmul(ps[:], wb1sb[kc, mc, tap][:],
                                         x_sb[kc][:, :, tap:tap + 100],
                                         start=first, stop=(kc == 3 and tap == 2))
                        first = False
                t_ = fr.tile([128, BATCH, 102], f32r, tag=f"base1_{mc}")
                dma_zero(t_[:, :, 0:1])
                dma_zero(t_[:, :, 101:102])
                nc.scalar.activation(t_[:, :, 1:101], ps[:], AF.Relu, bias=b1sb[mc][:])
                base1_sb.append(t_)

            # base = relu(conv1d(base1))  [128][4 b x 102]
            ps = frps.tile([128, BATCH, 100], f32, tag="ps_b2")
            first = True
            for kc in range(2):
                for tap in range(3):
                    nc.tensor.matmul(ps[:], wb2sb[kc, tap][:],
                                     base1_sb[kc][:, :, tap:tap + 100],
                                     start=first, stop=(kc == 1 and tap == 2))
                    first = False
            base_sb = fr.tile([128, BATCH, 102], f32)
            nc.vector.memset(base_sb[:], 0.0)
            nc.scalar.activation(base_sb[:, :, 1:101], ps[:], AF.Relu, bias=b2sb[0][:])

            # tem1 = relu(conv1d(base)) [2 chunks][4 x 102]
            tem1_sb = []
            for mc in range(2):
                ps = frps.tile([128, BATCH, 100], f32, tag="ps_t1")
                first = True
                for tap in range(3):
                    nc.tensor.matmul(ps[:], wt1sb[mc, tap][:],
                                     base_sb[:, :, tap:tap + 100],
                                     start=first, stop=(tap == 2))
                    first = False
                t_ = fr.tile([128, BATCH, 102], f32, tag=f"tem1_{mc}")
                nc.vector.memset(t_[:], 0.0)
                nc.scalar.activation(t_[:, :, 1:101], ps[:], AF.Relu, bias=bt1sb[mc][:])
                tem1_sb.append(t_)

            # tem_out = sigmoid(conv1d(tem1)) [2][4 x 100]
            ps = frps.tile([128, BATCH, 100], f32, tag="ps_t2")
            first = True
            for kc in range(2):
                for tap in range(3):
                    nc.tensor.matmul(ps[:2], wt2sb[kc, tap][:],
                                     tem1_sb[kc][:, :, tap:tap + 100],
                                     start=first, stop=(kc == 1 and tap == 2))
                    first = False
            temsb = fr.tile([2, BATCH, 100], f32)
            nc.scalar.activation(temsb[:], ps[:2], AF.Sigmoid, bias=bt2sb[:])
            for b in range(BATCH):
                nc.sync.dma_start(out=tem_d[b], in_=temsb[:, b, :])

            # baseT[b] = base[:, b, 1:101].T  -> [100 tt, 128 c]
            for b in range(BATCH):
                ps = frps.tile([128, 128], f32, tag="ps_tr")
                nc.tensor.transpose(ps[:100, :], base_sb[:, b, 1:101], ident[:])
                nc.vector.tensor_copy(baseT[b][:], ps[:100, :])

        # ---------------- phase A: sampling + conv3d + 1x1 over packed cols
        with tc.tile_pool(name="pem_sb", bufs=4) as pempool, \
             tc.tile_pool(name="y2", bufs=2) as y2pool, \
             tc.tile_pool(name="pa_ps", bufs=2, space="PSUM") as paps, \
             tc.tile_pool(name="y_ps", bufs=1, space="PSUM") as yps, \
             tc.tile_pool(name="z_ps", bufs=2, space="PSUM") as zps:
            for blk in range(NBLK):
                cs = blk * CB
                for b in range(BATCH):
                    y_ps = [yps.tile([128, CB], f32, tag=f"y{oc}", name=f"y_ps{oc}")
                            for oc in range(4)]
                    for n in range(NSMP):
                        pem_ps = paps.tile([128, CB], f32, tag="pem")
                        nc.tensor.matmul(pem_ps[:], baseT[b][:],
                                         wsmpsb[:, n, cs:cs + CB],
                                         start=True, stop=True)
                        pem_sb = pempool.tile([128, CB], bf16, tag="pem_sb")
                        nc.vector.tensor_copy(pem_sb[:], pem_ps[:])
                        for oc in range(4):
                            nc.tensor.matmul(
                                y_ps[oc][:],
                                w3rsb[n][:, oc * 128:(oc + 1) * 128],
                                pem_sb[:],
                                start=(n == 0), stop=(n == NSMP - 1))
                    y2 = []
                    for oc in range(4):
                        t_ = y2pool.tile([128, CB], bf16, tag=f"y2_{oc}")
                        nc.scalar.activation(t_[:], y_ps[oc][:], AF.Relu,
                                             bias=b3sb[oc][:])
                        y2.append(t_)
                    z_ps = zps.tile([128, CB], f32, tag="z")
                    for oc in range(4):
                        nc.tensor.matmul(z_ps[:], w1x1sb[oc][:], y2[oc][:],
                                         start=(oc == 0), stop=(oc == 3))
                    # cols are t-major (5 t-rows of 100 d) -> z1[b][:, 5blk:5blk+5, 1:101]
                    tr0 = cs // 100
                    nc.scalar.activation(
                        z1[b][:, tr0:tr0 + CB // 100, 1:101],
                        z_ps[:].rearrange("p (a c) -> p a c", a=CB // 100),
                        AF.Relu, bias=b2d1sb[0][:])

        # zero d-pad cols and mask out-of-range t rows
        for b in range(BATCH):
            dma_zero(z1[b][:, :, 0:1])
            dma_zero(z1[b][:, :, 101:102])
            for tr in range(TW):
                nc.vector.tensor_scalar_mul(z1[b][:, tr, 1:101],
                                            z1[b][:, tr, 1:101],
                                            tvalsb[:, tr:tr + 1])

        # ---------------- phase B: 3x3 conv + relu, 1x1 + sigmoid, DMA out
        DCH = [(0, 36), (36, 36), (72, 28)]
        with tc.tile_pool(name="z2", bufs=2) as z2pool, \
             tc.tile_pool(name="pb_ps", bufs=2, space="PSUM") as pbps, \
             tc.tile_pool(name="yo_ps", bufs=2, space="PSUM") as yops, \
             tc.tile_pool(name="yout", bufs=3) as yopool:
            for b in range(BATCH):
                for (d0, dn) in DCH:
                    # out cols ordered (t, d): rhs slices keep d contiguous
                    zz = pbps.tile([128, TC, dn], f32, tag=f"zz{dn}",
                                   name=f"zz{dn}")
                    for kh in (-1, 0, 1):        # d shift
                        for kw in (-1, 0, 1):    # t shift
                            tap = (kh + 1) * 3 + (kw + 1)
                            rhs = z1[b][:, 1 + kw:1 + kw + TC,
                                        d0 + 1 + kh:d0 + 1 + kh + dn]
                            nc.tensor.matmul(zz[:], w2d2sb[tap][:], rhs,
                                             start=(tap == 0), stop=(tap == 8))
                    z2 = z2pool.tile([128, TC, dn], f32, tag=f"z2{dn}",
                                     name=f"z2{dn}")
                    nc.scalar.activation(z2[:], zz[:], AF.Relu,
                                         bias=b2d2sb[0][:])
                    yo = yops.tile([128, TC, dn], f32, tag=f"yo{dn}",
                                   name=f"yo{dn}")
                    nc.tensor.matmul(yo[:2], w2d3sb[:], z2[:],
                                     start=True, stop=True)
                    yout = yopool.tile([2, TC, dn], f32, tag=f"yout{dn}",
                                       name=f"yout{dn}")
                    nc.scalar.activation(yout[:], yo[:2], AF.Sigmoid,
                                         bias=b2d3sb[:])
                    nc.sync.dma_start(out=y_d[b, :, :, d0:d0 + dn],
                                      in_=yout[:])

        z1_pool.release()
        persist.release()

    nc.compile()
    return nc


# ---------------------------------------------------------------- entry point
def _make_in_maps(inputs):
    import ml_dtypes
    wsmps, tvals, pr = _prep_host(inputs)
    shared = {
        "wb1t": pr["wb1t"], "b_base1": inputs["b_base1"],
        "wb2t": pr["wb2t"], "b_base2": inputs["b_base2"],
        "wt1t": pr["wt1t"], "b_tem1": inputs["b_tem1"],
        "wt2t": pr["wt2t"], "b_tem2": inputs["b_tem2"],
        "w3r": pr["w3r"], "b_c3d": inputs["b_c3d"],
        "w1x1t": pr["w1x1t"], "b_2d1": inputs["b_2d1"],
        "w2d2t": pr["w2d2t"], "b_2d2": inputs["b_2d2"],
        "w2d3t": pr["w2d3t"], "b_2d3": inputs["b_2d3"],
        "x": inputs["x"],
        "zeros": np.zeros(512, np.float32),
        "zerosh": np.zeros(512, ml_dtypes.bfloat16),
    }
    in_maps = []
    for r in range(NCORES):
        m = dict(shared)
        m["wsmp"] = wsmps[r]
        m["tval"] = tvals[r]
        in_maps.append(m)
    return in_maps


def kernel(**inputs):
    inputs = {k: np.asarray(v, dtype=np.float32) for k, v in inputs.items()}

    if "nc" not in _cache:
        _cache["nc"] = _build_program()
    nc = _cache["nc"]

    in_maps = _make_in_maps(inputs)

    from concourse.bass_utils import run_bass_kernel_spmd
    res = run_bass_kernel_spmd(nc, in_maps, list(range(NCORES)))
    _cache["last_res"] = res

    y = np.zeros((BATCH, 2, DPROP, T), np.float32)
    for r in range(NCORES):
        t0 = r * TC
        t1 = min(T, t0 + TC)
        yr = res.results[r]["y"].transpose(0, 1, 3, 2)   # -> [b, 2, d, t]
        y[:, :, :, t0:t1] = yr[:, :, :, : t1 - t0]
    tem = res.results[0]["tem"]
    return tem, y


# revision 20
# speedup vs baseline: 1.6507x; 1.6507x over previous
"""Trainium2 Bass kernel for the BMN-style nn module (nn_BMN_66683662238004).

Pipeline (per batch b):
  base = relu(conv1d(relu(conv1d(x))))            # [128, T]
  tem_out = sigmoid(conv1d(relu(conv1d(base))))   # [2, T]
  pem[c, n, d, t] = sum_tt base[c, tt] * Wsmp[tt, n, d, t]   (BM sampling)
  y1 = relu(conv3d(pem))   == per-(d,t) column: sum over n of W3_n[c,o] @ pem[c,n,(d,t)]
  z1 = relu(1x1(y1)); z2 = relu(3x3(z1)); y = sigmoid(1x1(z2))

Sharding: 8 cores, each owns a contiguous window of 13 t-columns (plus 1-col
halo each side for the 3x3 conv). Wsmp is precomputed on host (it is a
constant sparse interpolation matrix) and shipped pre-sliced per core.
All heavy matmuls run in fp32r (TF32-like) on the PE array.
"""

import os
import sys
import threading

import numpy as np

# ---------------------------------------------------------------- constants
T, NSMP, DPROP, EXPAND = 100, 32, 100, 0.5
FEAT, BATCH = 400, 4
NCORES = 8
TC = 13           # output t-columns per core (8*13 = 104 >= 100)
TW = TC + 2       # t-window incl halo
NCOLS = TW * 100  # packed phase-A columns per core (t-major, d-minor)
CB = 500          # phase-A column block (<=512 psum, >=256 for fp32r full rate)
NBLK = NCOLS // CB  # 3

_cache = {}


# ---------------------------------------------------------------- host math
def _smp_w4():
    """Faithful BMSampling weight, laid out [tt, n, t, d] float32."""
    ii = np.arange(T)                    # t (start index i)
    jj = np.arange(DPROP)                # d (duration index j)
    kk = np.arange(NSMP)
    J, I = np.meshgrid(jj, ii, indexing="ij")        # [d, t]
    valid_ij = J < np.minimum(T - 1 - I, DPROP)      # j < min(T-1-i, D); i<=T-2 implied
    length = (J + 1 - I).astype(np.float64)
    xmin_ext = I - length * EXPAND
    bin_size = (length + 2 * EXPAND * length) / (NSMP - 1)
    xp = xmin_ext[None] + kk[:, None, None] * bin_size[None]   # [n, d, t]
    ok = valid_ij[None] & (xp >= 0) & (xp <= T - 1)
    left = np.floor(xp).astype(np.int64)
    right = np.ceil(xp).astype(np.int64)
    wl = 1.0 - (xp - left)
    wr = 1.0 - (right - xp)
    w = np.zeros((T, NSMP, T, DPROP), np.float32)    # [tt, n, t, d]
    n_i, d_i, t_i = np.nonzero(ok)
    np.add.at(w, (left[ok], n_i, t_i, d_i), wl[ok])
    np.add.at(w, (right[ok], n_i, t_i, d_i), wr[ok])
    return w


def _prep_host(inputs):
    """Host-side constant prep: Wsmp slices + transposed weights."""
    import ml_dtypes
    bf16 = ml_dtypes.bfloat16
    w4 = _smp_w4()                                   # [tt, n, t, d]
    wsmps, tvals = [], []
    for r in range(NCORES):
        t0 = r * TC - 1
        sl = np.zeros((T, NSMP, TW, DPROP), np.float32)
        lo, hi = max(0, t0), min(T, t0 + TW)
        sl[:, :, lo - t0 : hi - t0, :] = w4[:, :, lo:hi, :]
        wsmps.append(np.ascontiguousarray(sl.reshape(T, NSMP, NCOLS).astype(bf16)))
        tv = np.zeros(TW, np.float32)
        tv[lo - t0 : hi - t0] = 1.0
        tvals.append(tv)

    pr = {
        "wb1t": np.ascontiguousarray(inputs["w_base1"].transpose(2, 1, 0)),  # [3,400,256]
        "wb2t": np.ascontiguousarray(inputs["w_base2"].transpose(2, 1, 0)),  # [3,256,128]
        "wt1t": np.ascontiguousarray(inputs["w_tem1"].transpose(2, 1, 0)),   # [3,128,256]
        "wt2t": np.ascontiguousarray(inputs["w_tem2"].transpose(2, 1, 0)),   # [3,256,2]
        "w3r": np.ascontiguousarray(inputs["w_c3d"].transpose(2, 1, 0)),     # [32,128,512]
        "w1x1t": np.ascontiguousarray(
            inputs["w_2d1"].reshape(128, 512).transpose(1, 0)),              # [512,128]
        "w2d2t": np.ascontiguousarray(
            inputs["w_2d2"].transpose(2, 3, 1, 0).reshape(9, 128, 128)),     # [kh*3+kw,c,o]
        "w2d3t": np.ascontiguousarray(inputs["w_2d3"].reshape(2, 128).transpose(1, 0)),
    }
    for k in ("w3r", "w1x1t", "w2d2t"):
        pr[k] = pr[k].astype(bf16)
    return wsmps, tvals, pr


# ---------------------------------------------------------------- device build
def _build_program():
    import concourse.bass as bass
    import concourse.tile as tile
    from concourse import bacc, mybir
    from concourse.masks import make_identity

    f32 = mybir.dt.float32
    f32r = mybir.dt.float32r
    bf16 = mybir.dt.bfloat16
    AF = mybir.ActivationFunctionType

    nc = bacc.Bacc("TRN2", target_bir_lowering=False, debug=False,
                   num_devices=NCORES)

    def din(name, shape):
        return nc.dram_tensor(name, shape, f32, kind="ExternalInput").ap()

    x_d = din("x", [BATCH, FEAT, T])
    zeros_d = din("zeros", [512])
    zerosh_d = nc.dram_tensor("zerosh", [512], mybir.dt.bfloat16,
                              kind="ExternalInput").ap()
    wsmp_d = nc.dram_tensor("wsmp", [T, NSMP, NCOLS], mybir.dt.bfloat16,
                            kind="ExternalInput").ap()
    tval_d = din("tval", [TW])
    wb1t_d = din("wb1t", [3, 400, 256])
    b1_d = din("b_base1", [256])
    wb2t_d = din("wb2t", [3, 256, 128])
    b2_d = din("b_base2", [128])
    wt1t_d = din("wt1t", [3, 128, 256])
    bt1_d = din("b_tem1", [256])
    wt2t_d = din("wt2t", [3, 256, 2])
    bt2_d = din("b_tem2", [2])
    w3r_d = nc.dram_tensor("w3r", [NSMP, 128, 512], mybir.dt.bfloat16,
                           kind="ExternalInput").ap()
    b3_d = din("b_c3d", [512])
    w1x1t_d = nc.dram_tensor("w1x1t", [512, 128], mybir.dt.bfloat16,
                             kind="ExternalInput").ap()
    b2d1_d = din("b_2d1", [128])
    w2d2t_d = nc.dram_tensor("w2d2t", [9, 128, 128], mybir.dt.bfloat16,
                             kind="ExternalInput").ap()
    b2d2_d = din("b_2d2", [128])
    w2d3t_d = din("w2d3t", [128, 2])
    b2d3_d = din("b_2d3", [2])

    # stored (t, d) on device; host transposes to (d, t)
    y_d = nc.dram_tensor("y", [BATCH, 2, TC, DPROP], f32, kind="ExternalOutput").ap()
    tem_d = nc.dram_tensor("tem", [BATCH, 2, T], f32, kind="ExternalOutput").ap()

    with tile.TileContext(nc) as tc:
        # ---------------- persistent pools
        persist = tc.alloc_tile_pool(name="persist", bufs=1)
        z1_pool = tc.alloc_tile_pool(name="z1", bufs=1)

        ident = persist.tile([128, 128], f32)
        make_identity(nc, ident)

        # biases as [p,1] tiles
        def bias_tiles(src, n_chunks, tag):
            ts = []
            for i in range(n_chunks):
                t_ = persist.tile([128, 1], f32, tag=f"{tag}{i}")
                nc.sync.dma_start(out=t_[:, 0], in_=src[i * 128:(i + 1) * 128])
                ts.append(t_)
            return ts

        b1sb = bias_tiles(b1_d, 2, "b1")
        b2sb = bias_tiles(b2_d, 1, "b2")
        bt1sb = bias_tiles(bt1_d, 2, "bt1")
        b3sb = bias_tiles(b3_d, 4, "b3")
        b2d1sb = bias_tiles(b2d1_d, 1, "b2d1")
        b2d2sb = bias_tiles(b2d2_d, 1, "b2d2")
        bt2sb = persist.tile([2, 1], f32)
        nc.sync.dma_start(out=bt2sb[:, 0], in_=bt2_d[:])
        b2d3sb = persist.tile([2, 1], f32)
        nc.sync.dma_start(out=b2d3sb[:, 0], in_=b2d3_d[:])

        tvalsb = persist.tile([128, TW], f32)
        nc.sync.dma_start(
            out=tvalsb[:],
            in_=bass.AP(tensor=tval_d.tensor, offset=tval_d.offset,
                        ap=[[0, 128], *tval_d.ap]))

        def dma_zero(out_ap):
            """Zero-fill an f32r/f32 tile region via DMA from the zeros input
            (memset ISA does not support float32r)."""
            dims = out_ap.shape
            if out_ap.dtype == bf16:
                srct = zerosh_d
            elif out_ap.dtype == f32:
                srct = zeros_d
            else:
                srct = zeros_d.bitcast(out_ap.dtype)
            ap = [[0, dims[0]]] + [[0, d] for d in dims[1:-1]] + [[1, dims[-1]]]
            nc.sync.dma_start(
                out=out_ap,
                in_=bass.AP(tensor=srct.tensor, offset=srct.offset, ap=ap))

        # w3r resident [32][128, 512], w1x1 [4][128,128], w2d2 taps, w2d3
        w3rsb = []
        for n in range(NSMP):
            w_ = persist.tile([128, 512], bf16, tag=f"w3r{n}")
            nc.sync.dma_start(out=w_[:], in_=w3r_d[n])
            w3rsb.append(w_)
        w1x1sb = []
        for oc in range(4):
            w_ = persist.tile([128, 128], bf16, tag=f"w1x1_{oc}")
            nc.sync.dma_start(out=w_[:], in_=w1x1t_d[oc * 128:(oc + 1) * 128, :])
            w1x1sb.append(w_)
        w2d2sb = []
        for tap in range(9):
            w_ = persist.tile([128, 128], bf16, tag=f"w2d2_{tap}")
            nc.sync.dma_start(out=w_[:], in_=w2d2t_d[tap])
            w2d2sb.append(w_)
        # wsmp fully resident in SBUF (bf16): one block per phase-A col block
        wsmpsb = persist.tile([100, NSMP, NCOLS], bf16)
        for blk in range(NBLK):
            nc.sync.dma_start(
                out=wsmpsb[:, :, blk * CB:(blk + 1) * CB],
                in_=wsmp_d[:, :, blk * CB:(blk + 1) * CB])
        w2d3sb = persist.tile([128, 2], f32)
        nc.sync.dma_start(out=w2d3sb[:], in_=w2d3t_d[:])

        # z1 maps: [128, TW trows, 102 dcols] per b  (d-pad cols 0 and 101)
        z1 = [z1_pool.tile([128, TW, 102], bf16, tag=f"z1b{b}", name=f"z1b{b}")
              for b in range(BATCH)]

        baseT = [persist.tile([100, 128], bf16, tag=f"baseT{b}", name=f"baseT{b}")
                 for b in range(BATCH)]

        # ---------------- front: conv1d stack + TEM + transposes
        with tc.tile_pool(name="front", bufs=1) as fr, \
             tc.tile_pool(name="front_ps", bufs=1, space="PSUM") as frps:
            # x -> sbuf [100, 4, 102] x4 chunks, t-padded
            x_sb = []
            for kc in range(4):
                t_ = fr.tile([100, BATCH, 102], f32r, tag=f"x{kc}")
                dma_zero(t_[:, :, 0:1])
                dma_zero(t_[:, :, 101:102])
                nc.sync.dma_start(
                    out=t_[:, :, 1:101],
                    in_=x_d[:, kc * 100:(kc + 1) * 100, :].rearrange("b c t -> c b t").bitcast(f32r))
                x_sb.append(t_)
            # conv weights
            wb1sb = {}
            for kc in range(4):
                for mc in range(2):
                    for tap in range(3):
                        w_ = fr.tile([100, 128], f32r, tag=f"wb1_{kc}_{mc}_{tap}")
                        nc.sync.dma_start(
                            out=w_[:],
                            in_=wb1t_d[tap, kc * 100:(kc + 1) * 100,
                                       mc * 128:(mc + 1) * 128].bitcast(f32r))
                        wb1sb[kc, mc, tap] = w_
            wb2sb = {}
            for kc in range(2):
                for tap in range(3):
                    w_ = fr.tile([128, 128], f32r, tag=f"wb2_{kc}_{tap}")
                    nc.sync.dma_start(
                        out=w_[:],
                        in_=wb2t_d[tap, kc * 128:(kc + 1) * 128, :].bitcast(f32r))
                    wb2sb[kc, tap] = w_
            wt1sb = {}
            for mc in range(2):
                for tap in range(3):
                    w_ = fr.tile([128, 128], f32, tag=f"wt1_{mc}_{tap}")
                    nc.sync.dma_start(
                        out=w_[:],
                        in_=wt1t_d[tap, :, mc * 128:(mc + 1) * 128])
                    wt1sb[mc, tap] = w_
            wt2sb = {}
            for kc in range(2):
                for tap in range(3):
                    w_ = fr.tile([128, 2], f32, tag=f"wt2_{kc}_{tap}")
                    nc.sync.dma_start(
                        out=w_[:],
                        in_=wt2t_d[tap, kc * 128:(kc + 1) * 128, :])
                    wt2sb[kc, tap] = w_

            # base1 = relu(conv1d(x))  [256 -> 2 chunks][100 t x 4 b]
            base1_sb = []
            for mc in range(2):
                ps = frps.tile([128, BATCH, 100], f32, tag="ps_b1")
                first = True
                for kc in range(4):
                    for tap in range(3):
                        nc.tensor.matmul(ps[:], wb1sb[kc, mc, tap][:],
                                         x_sb[kc][:, :, tap:tap + 100],
                                         start=first, stop=(kc == 3 and tap == 2))
                        first = False
                t_ = fr.tile([128, BATCH, 102], f32r, tag=f"base1_{mc}")
                dma_zero(t_[:, :, 0:1])
                dma_zero(t_[:, :, 101:102])
                nc.scalar.activation(t_[:, :, 1:101], ps[:], AF.Relu, bias=b1sb[mc][:])
                base1_sb.append(t_)

            # base = relu(conv1d(base1))  [128][4 b x 102]
            ps = frps.tile([128, BATCH, 100], f32, tag="ps_b2")
            first = True
            for kc in range(2):
                for tap in range(3):
                    nc.tensor.matmul(ps[:], wb2sb[kc, tap][:],
                                     base1_sb[kc][:, :, tap:tap + 100],
                                     start=first, stop=(kc == 1 and tap == 2))
                    first = False
            base_sb = fr.tile([128, BATCH, 102], f32)
            nc.vector.memset(base_sb[:], 0.0)
            nc.scalar.activation(base_sb[:, :, 1:101], ps[:], AF.Relu, bias=b2sb[0][:])

            # tem1 = relu(conv1d(base)) [2 chunks][4 x 102]
            tem1_sb = []
            for mc in range(2):
                ps = frps.tile([128, BATCH, 100], f32, tag="ps_t1")
                first = True
                for tap in range(3):
                    nc.tensor.matmul(ps[:], wt1sb[mc, tap][:],
                                     base_sb[:, :, tap:tap + 100],
                                     start=first, stop=(tap == 2))
                    first = False
                t_ = fr.tile([128, BATCH, 102], f32, tag=f"tem1_{mc}")
                nc.vector.memset(t_[:], 0.0)
                nc.scalar.activation(t_[:, :, 1:101], ps[:], AF.Relu, bias=bt1sb[mc][:])
                tem1_sb.append(t_)

            # tem_out = sigmoid(conv1d(tem1)) [2][4 x 100]
            ps = frps.tile([128, BATCH, 100], f32, tag="ps_t2")
            first = True
            for kc in range(2):
                for tap in range(3):
                    nc.tensor.matmul(ps[:2], wt2sb[kc, tap][:],
                                     tem1_sb[kc][:, :, tap:tap + 100],
                                     start=first, stop=(kc == 1 and tap == 2))
                    first = False
            temsb = fr.tile([2, BATCH, 100], f32)
            nc.scalar.activation(temsb[:], ps[:2], AF.Sigmoid, bias=bt2sb[:])
            for b in range(BATCH):
                nc.sync.dma_start(out=tem_d[b], in_=temsb[:, b, :])

            # baseT[b] = base[:, b, 1:101].T  -> [100 tt, 128 c]
            for b in range(BATCH):
                ps = frps.tile([128, 128], f32, tag="ps_tr")
                nc.tensor.transpose(ps[:100, :], base_sb[:, b, 1:101], ident[:])
                nc.vector.tensor_copy(baseT[b][:], ps[:100, :])

        # ---------------- phase A: sampling + conv3d + 1x1 over packed cols
        with tc.tile_pool(name="pem_sb", bufs=4) as pempool, \
             tc.tile_pool(name="y2", bufs=2) as y2pool, \
             tc.tile_pool(name="pa_ps", bufs=2, space="PSUM") as paps, \
             tc.tile_pool(name="y_ps", bufs=1, space="PSUM") as yps, \
             tc.tile_pool(name="z_ps", bufs=2, space="PSUM") as zps:
            for blk in range(NBLK):
                cs = blk * CB
                for b in range(BATCH):
                    y_ps = [yps.tile([128, CB], f32, tag=f"y{oc}", name=f"y_ps{oc}")
                            for oc in range(4)]
                    for n in range(NSMP):
                        pem_ps = paps.tile([128, CB], f32, tag="pem")
                        nc.tensor.matmul(pem_ps[:], baseT[b][:],
                                         wsmpsb[:, n, cs:cs + CB],
                                         start=True, stop=True)
                        pem_sb = pempool.tile([128, CB], bf16, tag="pem_sb")
                        nc.vector.tensor_copy(pem_sb[:], pem_ps[:])
                        for oc in range(4):
                            nc.tensor.matmul(
                                y_ps[oc][:],
                                w3rsb[n][:, oc * 128:(oc + 1) * 128],
                                pem_sb[:],
                                start=(n == 0), stop=(n == NSMP - 1))
                    y2 = []
                    for oc in range(4):
                        t_ = y2pool.tile([128, CB], bf16, tag=f"y2_{oc}")
                        nc.scalar.activation(t_[:], y_ps[oc][:], AF.Relu,
                                             bias=b3sb[oc][:])
                        y2.append(t_)
                    z_ps = zps.tile([128, CB], f32, tag="z")
                    for oc in range(4):
                        nc.tensor.matmul(z_ps[:], w1x1sb[oc][:], y2[oc][:],
                                         start=(oc == 0), stop=(oc == 3))
                    # cols are t-major (5 t-rows of 100 d) -> z1[b][:, 5blk:5blk+5, 1:101]
                    tr0 = cs // 100
                    nc.scalar.activation(
                        z1[b][:, tr0:tr0 + CB // 100, 1:101],
                        z_ps[:].rearrange("p (a c) -> p a c", a=CB // 100),
                        AF.Relu, bias=b2d1sb[0][:])

        # zero d-pad cols and mask out-of-range t rows
        for b in range(BATCH):
            nc.vector.memset(z1[b][:, :, 0:1], 0.0)
            nc.vector.memset(z1[b][:, :, 101:102], 0.0)
            for tr in range(TW):
                nc.vector.tensor_scalar_mul(z1[b][:, tr, 1:101],
                                            z1[b][:, tr, 1:101],
                                            tvalsb[:, tr:tr + 1])

        # ---------------- phase B: 3x3 conv + relu, 1x1 + sigmoid, DMA out
        DCH = [(0, 36), (36, 36), (72, 28)]
        with tc.tile_pool(name="z2", bufs=2) as z2pool, \
             tc.tile_pool(name="pb_ps", bufs=2, space="PSUM") as pbps, \
             tc.tile_pool(name="yo_ps", bufs=2, space="PSUM") as yops, \
             tc.tile_pool(name="yout", bufs=3) as yopool:
            for b in range(BATCH):
                for (d0, dn) in DCH:
                    # out cols ordered (t, d): rhs slices keep d contiguous
                    zz = pbps.tile([128, TC, dn], f32, tag=f"zz{dn}",
                                   name=f"zz{dn}")
                    for kh in (-1, 0, 1):        # d shift
                        for kw in (-1, 0, 1):    # t shift
                            tap = (kh + 1) * 3 + (kw + 1)
                            rhs = z1[b][:, 1 + kw:1 + kw + TC,
                                        d0 + 1 + kh:d0 + 1 + kh + dn]
                            nc.tensor.matmul(zz[:], w2d2sb[tap][:], rhs,
                                             start=(tap == 0), stop=(tap == 8))
                    z2 = z2pool.tile([128, TC, dn], f32, tag=f"z2{dn}",
                                     name=f"z2{dn}")
                    nc.scalar.activation(z2[:], zz[:], AF.Relu,
                                         bias=b2d2sb[0][:])
                    yo = yops.tile([128, TC, dn], f32, tag=f"yo{dn}",
                                   name=f"yo{dn}")
                    nc.tensor.matmul(yo[:2], w2d3sb[:], z2[:],
                                     start=True, stop=True)
                    yout = yopool.tile([2, TC, dn], f32, tag=f"yout{dn}",
                                       name=f"yout{dn}")
                    nc.scalar.activation(yout[:], yo[:2], AF.Sigmoid,
                                         bias=b2d3sb[:])
                    nc.sync.dma_start(out=y_d[b, :, :, d0:d0 + dn],
                                      in_=yout[:])

        z1_pool.release()
        persist.release()

    nc.compile()
    return nc


# ---------------------------------------------------------------- entry point
def _make_in_maps(inputs):
    import ml_dtypes
    wsmps, tvals, pr = _prep_host(inputs)
    shared = {
        "wb1t": pr["wb1t"], "b_base1": inputs["b_base1"],
        "wb2t": pr["wb2t"], "b_base2": inputs["b_base2"],
        "wt1t": pr["wt1t"], "b_tem1": inputs["b_tem1"],
        "wt2t": pr["wt2t"], "b_tem2": inputs["b_tem2"],
        "w3r": pr["w3r"], "b_c3d": inputs["b_c3d"],
        "w1x1t": pr["w1x1t"], "b_2d1": inputs["b_2d1"],
        "w2d2t": pr["w2d2t"], "b_2d2": inputs["b_2d2"],
        "w2d3t": pr["w2d3t"], "b_2d3": inputs["b_2d3"],
        "x": inputs["x"],
        "zeros": np.zeros(512, np.float32),
        "zerosh": np.zeros(512, ml_dtypes.bfloat16),
    }
    in_maps = []
    for r in range(NCORES):
        m = dict(shared)
        m["wsmp"] = wsmps[r]
        m["tval"] = tvals[r]
        in_maps.append(m)
    return in_maps


def kernel(**inputs):
    inputs = {k: np.asarray(v, dtype=np.float32) for k, v in inputs.items()}

    if "nc" not in _cache:
        _cache["nc"] = _build_program()
    nc = _cache["nc"]

    in_maps = _make_in_maps(inputs)

    from concourse.bass_utils import run_bass_kernel_spmd
    res = run_bass_kernel_spmd(nc, in_maps, list(range(NCORES)))
    _cache["last_res"] = res

    y = np.zeros((BATCH, 2, DPROP, T), np.float32)
    for r in range(NCORES):
        t0 = r * TC
        t1 = min(T, t0 + TC)
        yr = res.results[r]["y"].transpose(0, 1, 3, 2)   # -> [b, 2, d, t]
        y[:, :, :, t0:t1] = yr[:, :, :, : t1 - t0]
    tem = res.results[0]["tem"]
    return tem, y


# revision 22
# speedup vs baseline: 1.7563x; 1.0640x over previous
"""Trainium2 Bass kernel for the BMN-style nn module (nn_BMN_66683662238004).

Pipeline (per batch b):
  base = relu(conv1d(relu(conv1d(x))))            # [128, T]
  tem_out = sigmoid(conv1d(relu(conv1d(base))))   # [2, T]
  pem[c, n, d, t] = sum_tt base[c, tt] * Wsmp[tt, n, d, t]   (BM sampling)
  y1 = relu(conv3d(pem))   == per-(d,t) column: sum over n of W3_n[c,o] @ pem[c,n,(d,t)]
  z1 = relu(1x1(y1)); z2 = relu(3x3(z1)); y = sigmoid(1x1(z2))

Sharding: 8 cores, each owns a contiguous window of 13 t-columns (plus 1-col
halo each side for the 3x3 conv). Wsmp is precomputed on host (it is a
constant sparse interpolation matrix) and shipped pre-sliced per core.
All heavy matmuls run in fp32r (TF32-like) on the PE array.
"""

import os
import sys
import threading

import numpy as np

# ---------------------------------------------------------------- constants
T, NSMP, DPROP, EXPAND = 100, 32, 100, 0.5
FEAT, BATCH = 400, 4
NCORES = 8
TC = 13           # output t-columns per core (8*13 = 104 >= 100)
TW = TC + 2       # t-window incl halo
NCOLS = TW * 100  # packed phase-A columns per core (t-major, d-minor)
CB = 500          # phase-A column block (<=512 psum, >=256 for fp32r full rate)
NBLK = NCOLS // CB  # 3

_cache = {}


# ---------------------------------------------------------------- host math
def _smp_w4():
    """Faithful BMSampling weight, laid out [tt, n, t, d] float32."""
    ii = np.arange(T)                    # t (start index i)
    jj = np.arange(DPROP)                # d (duration index j)
    kk = np.arange(NSMP)
    J, I = np.meshgrid(jj, ii, indexing="ij")        # [d, t]
    valid_ij = J < np.minimum(T - 1 - I, DPROP)      # j < min(T-1-i, D); i<=T-2 implied
    length = (J + 1 - I).astype(np.float64)
    xmin_ext = I - length * EXPAND
    bin_size = (length + 2 * EXPAND * length) / (NSMP - 1)
    xp = xmin_ext[None] + kk[:, None, None] * bin_size[None]   # [n, d, t]
    ok = valid_ij[None] & (xp >= 0) & (xp <= T - 1)
    left = np.floor(xp).astype(np.int64)
    right = np.ceil(xp).astype(np.int64)
    wl = 1.0 - (xp - left)
    wr = 1.0 - (right - xp)
    w = np.zeros((T, NSMP, T, DPROP), np.float32)    # [tt, n, t, d]
    n_i, d_i, t_i = np.nonzero(ok)
    np.add.at(w, (left[ok], n_i, t_i, d_i), wl[ok])
    np.add.at(w, (right[ok], n_i, t_i, d_i), wr[ok])
    return w


def _prep_host(inputs):
    """Host-side constant prep: Wsmp slices + transposed weights."""
    import ml_dtypes
    bf16 = ml_dtypes.bfloat16
    w4 = _smp_w4()                                   # [tt, n, t, d]
    wsmps, tvals = [], []
    for r in range(NCORES):
        t0 = r * TC - 1
        sl = np.zeros((T, NSMP, TW, DPROP), np.float32)
        lo, hi = max(0, t0), min(T, t0 + TW)
        sl[:, :, lo - t0 : hi - t0, :] = w4[:, :, lo:hi, :]
        wsmps.append(np.ascontiguousarray(sl.reshape(T, NSMP, NCOLS).astype(bf16)))
        tv = np.zeros(TW, np.float32)
        tv[lo - t0 : hi - t0] = 1.0
        tvals.append(tv)

    pr = {
        "wb1t": np.ascontiguousarray(inputs["w_base1"].transpose(2, 1, 0)),  # [3,400,256]
        "wb2t": np.ascontiguousarray(inputs["w_base2"].transpose(2, 1, 0)),  # [3,256,128]
        "wt1t": np.ascontiguousarray(inputs["w_tem1"].transpose(2, 1, 0)),   # [3,128,256]
        "wt2t": np.ascontiguousarray(inputs["w_tem2"].transpose(2, 1, 0)),   # [3,256,2]
        "w3r": np.ascontiguousarray(inputs["w_c3d"].transpose(2, 1, 0)),     # [32,128,512]
        "w1x1t": np.ascontiguousarray(
            inputs["w_2d1"].reshape(128, 512).transpose(1, 0)),              # [512,128]
        "w2d2t": np.ascontiguousarray(
            inputs["w_2d2"].transpose(2, 3, 1, 0).reshape(9, 128, 128)),     # [kh*3+kw,c,o]
        "w2d3t": np.ascontiguousarray(inputs["w_2d3"].reshape(2, 128).transpose(1, 0)),
    }
    for k in ("w3r", "w1x1t", "w2d2t"):
        pr[k] = pr[k].astype(bf16)
    return wsmps, tvals, pr


# ---------------------------------------------------------------- device build
def _build_program():
    import concourse.bass as bass
    import concourse.tile as tile
    from concourse import bacc, mybir
    from concourse.masks import make_identity

    f32 = mybir.dt.float32
    f32r = mybir.dt.float32r
    bf16 = mybir.dt.bfloat16
    AF = mybir.ActivationFunctionType

    nc = bacc.Bacc("TRN2", target_bir_lowering=False, debug=False,
                   num_devices=NCORES)

    def din(name, shape):
        return nc.dram_tensor(name, shape, f32, kind="ExternalInput").ap()

    x_d = din("x", [BATCH, FEAT, T])
    zeros_d = din("zeros", [512])
    zerosh_d = nc.dram_tensor("zerosh", [512], mybir.dt.bfloat16,
                              kind="ExternalInput").ap()
    wsmp_d = nc.dram_tensor("wsmp", [T, NSMP, NCOLS], mybir.dt.bfloat16,
                            kind="ExternalInput").ap()
    tval_d = din("tval", [TW])
    wb1t_d = din("wb1t", [3, 400, 256])
    b1_d = din("b_base1", [256])
    wb2t_d = din("wb2t", [3, 256, 128])
    b2_d = din("b_base2", [128])
    wt1t_d = din("wt1t", [3, 128, 256])
    bt1_d = din("b_tem1", [256])
    wt2t_d = din("wt2t", [3, 256, 2])
    bt2_d = din("b_tem2", [2])
    w3r_d = nc.dram_tensor("w3r", [NSMP, 128, 512], mybir.dt.bfloat16,
                           kind="ExternalInput").ap()
    b3_d = din("b_c3d", [512])
    w1x1t_d = nc.dram_tensor("w1x1t", [512, 128], mybir.dt.bfloat16,
                             kind="ExternalInput").ap()
    b2d1_d = din("b_2d1", [128])
    w2d2t_d = nc.dram_tensor("w2d2t", [9, 128, 128], mybir.dt.bfloat16,
                             kind="ExternalInput").ap()
    b2d2_d = din("b_2d2", [128])
    w2d3t_d = din("w2d3t", [128, 2])
    b2d3_d = din("b_2d3", [2])

    # stored (t, d) on device; host transposes to (d, t)
    y_d = nc.dram_tensor("y", [BATCH, 2, TC, DPROP], f32, kind="ExternalOutput").ap()
    tem_d = nc.dram_tensor("tem", [BATCH, 2, T], f32, kind="ExternalOutput").ap()

    with tile.TileContext(nc) as tc:
        # ---------------- persistent pools
        persist = tc.alloc_tile_pool(name="persist", bufs=1)
        z1_pool = tc.alloc_tile_pool(name="z1", bufs=1)

        ident = persist.tile([128, 128], f32)
        make_identity(nc, ident)

        # biases as [p,1] tiles
        def bias_tiles(src, n_chunks, tag):
            ts = []
            for i in range(n_chunks):
                t_ = persist.tile([128, 1], f32, tag=f"{tag}{i}")
                nc.sync.dma_start(out=t_[:, 0], in_=src[i * 128:(i + 1) * 128])
                ts.append(t_)
            return ts

        b1sb = bias_tiles(b1_d, 2, "b1")
        b2sb = bias_tiles(b2_d, 1, "b2")
        bt1sb = bias_tiles(bt1_d, 2, "bt1")
        b3sb = bias_tiles(b3_d, 4, "b3")
        b2d1sb = bias_tiles(b2d1_d, 1, "b2d1")
        b2d2sb = bias_tiles(b2d2_d, 1, "b2d2")
        bt2sb = persist.tile([2, 1], f32)
        nc.sync.dma_start(out=bt2sb[:, 0], in_=bt2_d[:])
        b2d3sb = persist.tile([2, 1], f32)
        nc.sync.dma_start(out=b2d3sb[:, 0], in_=b2d3_d[:])

        tvalsb = persist.tile([128, TW], f32)
        nc.sync.dma_start(
            out=tvalsb[:],
            in_=bass.AP(tensor=tval_d.tensor, offset=tval_d.offset,
                        ap=[[0, 128], *tval_d.ap]))

        def dma_zero(out_ap):
            """Zero-fill an f32r/f32 tile region via DMA from the zeros input
            (memset ISA does not support float32r)."""
            dims = out_ap.shape
            if out_ap.dtype == bf16:
                srct = zerosh_d
            elif out_ap.dtype == f32:
                srct = zeros_d
            else:
                srct = zeros_d.bitcast(out_ap.dtype)
            ap = [[0, dims[0]]] + [[0, d] for d in dims[1:-1]] + [[1, dims[-1]]]
            nc.sync.dma_start(
                out=out_ap,
                in_=bass.AP(tensor=srct.tensor, offset=srct.offset, ap=ap))

        # w3r resident [32][128, 512], w1x1 [4][128,128], w2d2 taps, w2d3
        w3rsb = []
        for n in range(NSMP):
            w_ = persist.tile([128, 512], bf16, tag=f"w3r{n}")
            nc.gpsimd.dma_start(out=w_[:], in_=w3r_d[n])
            w3rsb.append(w_)
        w1x1sb = []
        for oc in range(4):
            w_ = persist.tile([128, 128], bf16, tag=f"w1x1_{oc}")
            nc.gpsimd.dma_start(out=w_[:], in_=w1x1t_d[oc * 128:(oc + 1) * 128, :])
            w1x1sb.append(w_)
        w2d2sb = []
        for tap in range(9):
            w_ = persist.tile([128, 128], bf16, tag=f"w2d2_{tap}")
            nc.gpsimd.dma_start(out=w_[:], in_=w2d2t_d[tap])
            w2d2sb.append(w_)
        # wsmp fully resident in SBUF (bf16); fine-grained DMAs on the
        # scalar-engine HWDGE queue so they don't block the front's loads
        wsmpsb = persist.tile([100, NSMP, NCOLS], bf16)
        for blk in range(NBLK):
            for n in range(NSMP):
                nc.scalar.dma_start(
                    out=wsmpsb[:, n, blk * CB:(blk + 1) * CB],
                    in_=wsmp_d[:, n, blk * CB:(blk + 1) * CB])
        w2d3sb = persist.tile([128, 2], f32)
        nc.sync.dma_start(out=w2d3sb[:], in_=w2d3t_d[:])

        # z1 maps: [128, TW trows, 102 dcols] per b  (d-pad cols 0 and 101)
        z1 = [z1_pool.tile([128, TW, 102], bf16, tag=f"z1b{b}", name=f"z1b{b}")
              for b in range(BATCH)]

        baseT = [persist.tile([100, 128], bf16, tag=f"baseT{b}", name=f"baseT{b}")
                 for b in range(BATCH)]

        # ---------------- front: conv1d stack + TEM + transposes
        with tc.tile_pool(name="front", bufs=1) as fr, \
             tc.tile_pool(name="front_ps", bufs=1, space="PSUM") as frps:
            # x -> sbuf [100, 4, 102] x4 chunks, t-padded
            x_sb = []
            for kc in range(4):
                t_ = fr.tile([100, BATCH, 102], f32r, tag=f"x{kc}")
                dma_zero(t_[:, :, 0:1])
                dma_zero(t_[:, :, 101:102])
                nc.sync.dma_start(
                    out=t_[:, :, 1:101],
                    in_=x_d[:, kc * 100:(kc + 1) * 100, :].rearrange("b c t -> c b t").bitcast(f32r))
                x_sb.append(t_)
            # conv weights
            wb1sb = {}
            for kc in range(4):
                for mc in range(2):
                    for tap in range(3):
                        w_ = fr.tile([100, 128], f32r, tag=f"wb1_{kc}_{mc}_{tap}")
                        nc.sync.dma_start(
                            out=w_[:],
                            in_=wb1t_d[tap, kc * 100:(kc + 1) * 100,
                                       mc * 128:(mc + 1) * 128].bitcast(f32r))
                        wb1sb[kc, mc, tap] = w_
            wb2sb = {}
            for kc in range(2):
                for tap in range(3):
                    w_ = fr.tile([128, 128], f32r, tag=f"wb2_{kc}_{tap}")
                    nc.sync.dma_start(
                        out=w_[:],
                        in_=wb2t_d[tap, kc * 128:(kc + 1) * 128, :].bitcast(f32r))
                    wb2sb[kc, tap] = w_
            wt1sb = {}
            for mc in range(2):
                for tap in range(3):
                    w_ = fr.tile([128, 128], f32, tag=f"wt1_{mc}_{tap}")
                    nc.sync.dma_start(
                        out=w_[:],
                        in_=wt1t_d[tap, :, mc * 128:(mc + 1) * 128])
                    wt1sb[mc, tap] = w_
            wt2sb = {}
            for kc in range(2):
                for tap in range(3):
                    w_ = fr.tile([128, 2], f32, tag=f"wt2_{kc}_{tap}")
                    nc.sync.dma_start(
                        out=w_[:],
                        in_=wt2t_d[tap, kc * 128:(kc + 1) * 128, :])
                    wt2sb[kc, tap] = w_

            # base1 = relu(conv1d(x))  [256 -> 2 chunks][100 t x 4 b]
            base1_sb = []
            for mc in range(2):
                ps = frps.tile([128, BATCH, 100], f32, tag="ps_b1")
                first = True
                for kc in range(4):
                    for tap in range(3):
                        nc.tensor.matmul(ps[:], wb1sb[kc, mc, tap][:],
                                         x_sb[kc][:, :, tap:tap + 100],
                                         start=first, stop=(kc == 3 and tap == 2))
                        first = False
                t_ = fr.tile([128, BATCH, 102], f32r, tag=f"base1_{mc}")
                dma_zero(t_[:, :, 0:1])
                dma_zero(t_[:, :, 101:102])
                nc.scalar.activation(t_[:, :, 1:101], ps[:], AF.Relu, bias=b1sb[mc][:])
                base1_sb.append(t_)

            # base = relu(conv1d(base1))  [128][4 b x 102]
            ps = frps.tile([128, BATCH, 100], f32, tag="ps_b2")
            first = True
            for kc in range(2):
                for tap in range(3):
                    nc.tensor.matmul(ps[:], wb2sb[kc, tap][:],
                                     base1_sb[kc][:, :, tap:tap + 100],
                                     start=first, stop=(kc == 1 and tap == 2))
                    first = False
            base_sb = fr.tile([128, BATCH, 102], f32)
            nc.vector.memset(base_sb[:], 0.0)
            nc.scalar.activation(base_sb[:, :, 1:101], ps[:], AF.Relu, bias=b2sb[0][:])

            # tem1 = relu(conv1d(base)) [2 chunks][4 x 102]
            tem1_sb = []
            for mc in range(2):
                ps = frps.tile([128, BATCH, 100], f32, tag="ps_t1")
                first = True
                for tap in range(3):
                    nc.tensor.matmul(ps[:], wt1sb[mc, tap][:],
                                     base_sb[:, :, tap:tap + 100],
                                     start=first, stop=(tap == 2))
                    first = False
                t_ = fr.tile([128, BATCH, 102], f32, tag=f"tem1_{mc}")
                nc.vector.memset(t_[:], 0.0)
                nc.scalar.activation(t_[:, :, 1:101], ps[:], AF.Relu, bias=bt1sb[mc][:])
                tem1_sb.append(t_)

            # tem_out = sigmoid(conv1d(tem1)) [2][4 x 100]
            ps = frps.tile([128, BATCH, 100], f32, tag="ps_t2")
            first = True
            for kc in range(2):
                for tap in range(3):
                    nc.tensor.matmul(ps[:2], wt2sb[kc, tap][:],
                                     tem1_sb[kc][:, :, tap:tap + 100],
                                     start=first, stop=(kc == 1 and tap == 2))
                    first = False
            temsb = fr.tile([2, BATCH, 100], f32)
            nc.scalar.activation(temsb[:], ps[:2], AF.Sigmoid, bias=bt2sb[:])
            for b in range(BATCH):
                nc.sync.dma_start(out=tem_d[b], in_=temsb[:, b, :])

            # baseT[b] = base[:, b, 1:101].T  -> [100 tt, 128 c]
            for b in range(BATCH):
                ps = frps.tile([128, 128], f32, tag="ps_tr")
                nc.tensor.transpose(ps[:100, :], base_sb[:, b, 1:101], ident[:])
                nc.vector.tensor_copy(baseT[b][:], ps[:100, :])

        # ---------------- phase A: sampling + conv3d + 1x1 over packed cols
        with tc.tile_pool(name="pem_sb", bufs=4) as pempool, \
             tc.tile_pool(name="y2", bufs=2) as y2pool, \
             tc.tile_pool(name="pa_ps", bufs=2, space="PSUM") as paps, \
             tc.tile_pool(name="y_ps", bufs=1, space="PSUM") as yps, \
             tc.tile_pool(name="z_ps", bufs=2, space="PSUM") as zps:
            for blk in range(NBLK):
                cs = blk * CB
                for b in range(BATCH):
                    y_ps = [yps.tile([128, CB], f32, tag=f"y{oc}", name=f"y_ps{oc}")
                            for oc in range(4)]
                    for n in range(NSMP):
                        pem_ps = paps.tile([128, CB], f32, tag="pem")
                        nc.tensor.matmul(pem_ps[:], baseT[b][:],
                                         wsmpsb[:, n, cs:cs + CB],
                                         start=True, stop=True)
                        pem_sb = pempool.tile([128, CB], bf16, tag="pem_sb")
                        nc.vector.tensor_copy(pem_sb[:], pem_ps[:])
                        for oc in range(4):
                            nc.tensor.matmul(
                                y_ps[oc][:],
                                w3rsb[n][:, oc * 128:(oc + 1) * 128],
                                pem_sb[:],
                                start=(n == 0), stop=(n == NSMP - 1))
                    y2 = []
                    for oc in range(4):
                        t_ = y2pool.tile([128, CB], bf16, tag=f"y2_{oc}")
                        nc.scalar.activation(t_[:], y_ps[oc][:], AF.Relu,
                                             bias=b3sb[oc][:])
                        y2.append(t_)
                    z_ps = zps.tile([128, CB], f32, tag="z")
                    for oc in range(4):
                        nc.tensor.matmul(z_ps[:], w1x1sb[oc][:], y2[oc][:],
                                         start=(oc == 0), stop=(oc == 3))
                    # cols are t-major (5 t-rows of 100 d) -> z1[b][:, 5blk:5blk+5, 1:101]
                    tr0 = cs // 100
                    nc.scalar.activation(
                        z1[b][:, tr0:tr0 + CB // 100, 1:101],
                        z_ps[:].rearrange("p (a c) -> p a c", a=CB // 100),
                        AF.Relu, bias=b2d1sb[0][:])

        # zero d-pad cols and mask out-of-range t rows
        for b in range(BATCH):
            nc.vector.memset(z1[b][:, :, 0:1], 0.0)
            nc.vector.memset(z1[b][:, :, 101:102], 0.0)
            for tr in range(TW):
                nc.vector.tensor_scalar_mul(z1[b][:, tr, 1:101],
                                            z1[b][:, tr, 1:101],
                                            tvalsb[:, tr:tr + 1])

        # ---------------- phase B: 3x3 conv + relu, 1x1 + sigmoid, DMA out
        DCH = [(0, 36), (36, 36), (72, 28)]
        with tc.tile_pool(name="z2", bufs=2) as z2pool, \
             tc.tile_pool(name="pb_ps", bufs=2, space="PSUM") as pbps, \
             tc.tile_pool(name="yo_ps", bufs=2, space="PSUM") as yops, \
             tc.tile_pool(name="yout", bufs=3) as yopool:
            for b in range(BATCH):
                for (d0, dn) in DCH:
                    # out cols ordered (t, d): rhs slices keep d contiguous
                    zz = pbps.tile([128, TC, dn], f32, tag=f"zz{dn}",
                                   name=f"zz{dn}")
                    for kh in (-1, 0, 1):        # d shift
                        for kw in (-1, 0, 1):    # t shift
                            tap = (kh + 1) * 3 + (kw + 1)
                            rhs = z1[b][:, 1 + kw:1 + kw + TC,
                                        d0 + 1 + kh:d0 + 1 + kh + dn]
                            nc.tensor.matmul(zz[:], w2d2sb[tap][:], rhs,
                                             start=(tap == 0), stop=(tap == 8))
                    z2 = z2pool.tile([128, TC, dn], f32, tag=f"z2{dn}",
                                     name=f"z2{dn}")
                    nc.scalar.activation(z2[:], zz[:], AF.Relu,
                                         bias=b2d2sb[0][:])
                    yo = yops.tile([128, TC, dn], f32, tag=f"yo{dn}",
                                   name=f"yo{dn}")
                    nc.tensor.matmul(yo[:2], w2d3sb[:], z2[:],
                                     start=True, stop=True)
                    yout = yopool.tile([2, TC, dn], f32, tag=f"yout{dn}",
                                       name=f"yout{dn}")
                    nc.scalar.activation(yout[:], yo[:2], AF.Sigmoid,
                                         bias=b2d3sb[:])
                    nc.sync.dma_start(out=y_d[b, :, :, d0:d0 + dn],
                                      in_=yout[:])

        z1_pool.release()
        persist.release()

    nc.compile()
    return nc


# ---------------------------------------------------------------- entry point
def _make_in_maps(inputs):
    import ml_dtypes
    wsmps, tvals, pr = _prep_host(inputs)
    shared = {
        "wb1t": pr["wb1t"], "b_base1": inputs["b_base1"],
        "wb2t": pr["wb2t"], "b_base2": inputs["b_base2"],
        "wt1t": pr["wt1t"], "b_tem1": inputs["b_tem1"],
        "wt2t": pr["wt2t"], "b_tem2": inputs["b_tem2"],
        "w3r": pr["w3r"], "b_c3d": inputs["b_c3d"],
        "w1x1t": pr["w1x1t"], "b_2d1": inputs["b_2d1"],
        "w2d2t": pr["w2d2t"], "b_2d2": inputs["b_2d2"],
        "w2d3t": pr["w2d3t"], "b_2d3": inputs["b_2d3"],
        "x": inputs["x"],
        "zeros": np.zeros(512, np.float32),
        "zerosh": np.zeros(512, ml_dtypes.bfloat16),
    }
    in_maps = []
    for r in range(NCORES):
        m = dict(shared)
        m["wsmp"] = wsmps[r]
        m["tval"] = tvals[r]
        in_maps.append(m)
    return in_maps


def kernel(**inputs):
    inputs = {k: np.asarray(v, dtype=np.float32) for k, v in inputs.items()}

    if "nc" not in _cache:
        _cache["nc"] = _build_program()
    nc = _cache["nc"]

    in_maps = _make_in_maps(inputs)

    from concourse.bass_utils import run_bass_kernel_spmd
    res = run_bass_kernel_spmd(nc, in_maps, list(range(NCORES)))
    _cache["last_res"] = res

    y = np.zeros((BATCH, 2, DPROP, T), np.float32)
    for r in range(NCORES):
        t0 = r * TC
        t1 = min(T, t0 + TC)
        yr = res.results[r]["y"].transpose(0, 1, 3, 2)   # -> [b, 2, d, t]
        y[:, :, :, t0:t1] = yr[:, :, :, : t1 - t0]
    tem = res.results[0]["tem"]
    return tem, y


# revision 23
# speedup vs baseline: 1.7657x; 1.0054x over previous
"""Trainium2 Bass kernel for the BMN-style nn module (nn_BMN_66683662238004).

Pipeline (per batch b):
  base = relu(conv1d(relu(conv1d(x))))            # [128, T]
  tem_out = sigmoid(conv1d(relu(conv1d(base))))   # [2, T]
  pem[c, n, d, t] = sum_tt base[c, tt] * Wsmp[tt, n, d, t]   (BM sampling)
  y1 = relu(conv3d(pem))   == per-(d,t) column: sum over n of W3_n[c,o] @ pem[c,n,(d,t)]
  z1 = relu(1x1(y1)); z2 = relu(3x3(z1)); y = sigmoid(1x1(z2))

Sharding: 8 cores, each owns a contiguous window of 13 t-columns (plus 1-col
halo each side for the 3x3 conv). Wsmp is precomputed on host (it is a
constant sparse interpolation matrix) and shipped pre-sliced per core.
All heavy matmuls run in fp32r (TF32-like) on the PE array.
"""

import os
import sys
import threading

import numpy as np

# ---------------------------------------------------------------- constants
T, NSMP, DPROP, EXPAND = 100, 32, 100, 0.5
FEAT, BATCH = 400, 4
NCORES = 8
TC = 13           # output t-columns per core (8*13 = 104 >= 100)
TW = TC + 2       # t-window incl halo
NCOLS = TW * 100  # packed phase-A columns per core (t-major, d-minor)
CB = 500          # phase-A column block (<=512 psum, >=256 for fp32r full rate)
NBLK = NCOLS // CB  # 3

_cache = {}


# ---------------------------------------------------------------- host math
def _smp_w4():
    """Faithful BMSampling weight, laid out [tt, n, t, d] float32."""
    ii = np.arange(T)                    # t (start index i)
    jj = np.arange(DPROP)                # d (duration index j)
    kk = np.arange(NSMP)
    J, I = np.meshgrid(jj, ii, indexing="ij")        # [d, t]
    valid_ij = J < np.minimum(T - 1 - I, DPROP)      # j < min(T-1-i, D); i<=T-2 implied
    length = (J + 1 - I).astype(np.float64)
    xmin_ext = I - length * EXPAND
    bin_size = (length + 2 * EXPAND * length) / (NSMP - 1)
    xp = xmin_ext[None] + kk[:, None, None] * bin_size[None]   # [n, d, t]
    ok = valid_ij[None] & (xp >= 0) & (xp <= T - 1)
    left = np.floor(xp).astype(np.int64)
    right = np.ceil(xp).astype(np.int64)
    wl = 1.0 - (xp - left)
    wr = 1.0 - (right - xp)
    w = np.zeros((T, NSMP, T, DPROP), np.float32)    # [tt, n, t, d]
    n_i, d_i, t_i = np.nonzero(ok)
    np.add.at(w, (left[ok], n_i, t_i, d_i), wl[ok])
    np.add.at(w, (right[ok], n_i, t_i, d_i), wr[ok])
    return w


def _prep_host(inputs):
    """Host-side constant prep: Wsmp slices + transposed weights."""
    import ml_dtypes
    bf16 = ml_dtypes.bfloat16
    w4 = _smp_w4()                                   # [tt, n, t, d]
    wsmps, tvals = [], []
    for r in range(NCORES):
        t0 = r * TC - 1
        sl = np.zeros((T, NSMP, TW, DPROP), np.float32)
        lo, hi = max(0, t0), min(T, t0 + TW)
        sl[:, :, lo - t0 : hi - t0, :] = w4[:, :, lo:hi, :]
        wsmps.append(np.ascontiguousarray(sl.reshape(T, NSMP, NCOLS).astype(bf16)))
        tv = np.zeros(TW, np.float32)
        tv[lo - t0 : hi - t0] = 1.0
        tvals.append(tv)

    pr = {
        "wb1t": np.ascontiguousarray(inputs["w_base1"].transpose(2, 1, 0)),  # [3,400,256]
        "wb2t": np.ascontiguousarray(inputs["w_base2"].transpose(2, 1, 0)),  # [3,256,128]
        "wt1t": np.ascontiguousarray(inputs["w_tem1"].transpose(2, 1, 0)),   # [3,128,256]
        "wt2t": np.ascontiguousarray(inputs["w_tem2"].transpose(2, 1, 0)),   # [3,256,2]
        "w3r": np.ascontiguousarray(inputs["w_c3d"].transpose(2, 1, 0)),     # [32,128,512]
        "w1x1t": np.ascontiguousarray(
            inputs["w_2d1"].reshape(128, 512).transpose(1, 0)),              # [512,128]
        "w2d2t": np.ascontiguousarray(
            inputs["w_2d2"].transpose(2, 3, 1, 0).reshape(9, 128, 128)),     # [kh*3+kw,c,o]
        "w2d3t": np.ascontiguousarray(inputs["w_2d3"].reshape(2, 128).transpose(1, 0)),
    }
    for k in ("w3r", "w1x1t", "w2d2t"):
        pr[k] = pr[k].astype(bf16)
    return wsmps, tvals, pr


# ---------------------------------------------------------------- device build
def _build_program():
    import concourse.bass as bass
    import concourse.tile as tile
    from concourse import bacc, mybir
    from concourse.masks import make_identity

    f32 = mybir.dt.float32
    f32r = mybir.dt.float32r
    bf16 = mybir.dt.bfloat16
    AF = mybir.ActivationFunctionType

    nc = bacc.Bacc("TRN2", target_bir_lowering=False, debug=False,
                   num_devices=NCORES)

    def din(name, shape):
        return nc.dram_tensor(name, shape, f32, kind="ExternalInput").ap()

    x_d = din("x", [BATCH, FEAT, T])
    zeros_d = din("zeros", [512])
    zerosh_d = nc.dram_tensor("zerosh", [512], mybir.dt.bfloat16,
                              kind="ExternalInput").ap()
    wsmp_d = nc.dram_tensor("wsmp", [T, NSMP, NCOLS], mybir.dt.bfloat16,
                            kind="ExternalInput").ap()
    tval_d = din("tval", [TW])
    wb1t_d = din("wb1t", [3, 400, 256])
    b1_d = din("b_base1", [256])
    wb2t_d = din("wb2t", [3, 256, 128])
    b2_d = din("b_base2", [128])
    wt1t_d = din("wt1t", [3, 128, 256])
    bt1_d = din("b_tem1", [256])
    wt2t_d = din("wt2t", [3, 256, 2])
    bt2_d = din("b_tem2", [2])
    w3r_d = nc.dram_tensor("w3r", [NSMP, 128, 512], mybir.dt.bfloat16,
                           kind="ExternalInput").ap()
    b3_d = din("b_c3d", [512])
    w1x1t_d = nc.dram_tensor("w1x1t", [512, 128], mybir.dt.bfloat16,
                             kind="ExternalInput").ap()
    b2d1_d = din("b_2d1", [128])
    w2d2t_d = nc.dram_tensor("w2d2t", [9, 128, 128], mybir.dt.bfloat16,
                             kind="ExternalInput").ap()
    b2d2_d = din("b_2d2", [128])
    w2d3t_d = din("w2d3t", [128, 2])
    b2d3_d = din("b_2d3", [2])

    # stored (t, d) on device; host transposes to (d, t)
    y_d = nc.dram_tensor("y", [BATCH, 2, TC, DPROP], f32, kind="ExternalOutput").ap()
    tem_d = nc.dram_tensor("tem", [BATCH, 2, T], f32, kind="ExternalOutput").ap()

    with tile.TileContext(nc) as tc:
        # ---------------- persistent pools
        persist = tc.alloc_tile_pool(name="persist", bufs=1)
        z1_pool = tc.alloc_tile_pool(name="z1", bufs=1)

        ident = persist.tile([128, 128], f32)
        make_identity(nc, ident)

        # biases as [p,1] tiles
        def bias_tiles(src, n_chunks, tag):
            ts = []
            for i in range(n_chunks):
                t_ = persist.tile([128, 1], f32, tag=f"{tag}{i}")
                nc.gpsimd.dma_start(out=t_[:, 0], in_=src[i * 128:(i + 1) * 128])
                ts.append(t_)
            return ts

        b1sb = bias_tiles(b1_d, 2, "b1")
        b2sb = bias_tiles(b2_d, 1, "b2")
        bt1sb = bias_tiles(bt1_d, 2, "bt1")
        b3sb = bias_tiles(b3_d, 4, "b3")
        b2d1sb = bias_tiles(b2d1_d, 1, "b2d1")
        b2d2sb = bias_tiles(b2d2_d, 1, "b2d2")
        bt2sb = persist.tile([2, 1], f32)
        nc.sync.dma_start(out=bt2sb[:, 0], in_=bt2_d[:])
        b2d3sb = persist.tile([2, 1], f32)
        nc.sync.dma_start(out=b2d3sb[:, 0], in_=b2d3_d[:])

        tvalsb = persist.tile([128, TW], f32)
        nc.sync.dma_start(
            out=tvalsb[:],
            in_=bass.AP(tensor=tval_d.tensor, offset=tval_d.offset,
                        ap=[[0, 128], *tval_d.ap]))

        def dma_zero(out_ap):
            """Zero-fill an f32r/f32 tile region via DMA from the zeros input
            (memset ISA does not support float32r)."""
            dims = out_ap.shape
            if out_ap.dtype == bf16:
                srct = zerosh_d
            elif out_ap.dtype == f32:
                srct = zeros_d
            else:
                srct = zeros_d.bitcast(out_ap.dtype)
            ap = [[0, dims[0]]] + [[0, d] for d in dims[1:-1]] + [[1, dims[-1]]]
            nc.sync.dma_start(
                out=out_ap,
                in_=bass.AP(tensor=srct.tensor, offset=srct.offset, ap=ap))

        # w3r resident [32][128, 512], w1x1 [4][128,128], w2d2 taps, w2d3
        w3rsb = []
        for n in range(NSMP):
            w_ = persist.tile([128, 512], bf16, tag=f"w3r{n}")
            nc.gpsimd.dma_start(out=w_[:], in_=w3r_d[n])
            w3rsb.append(w_)
        w1x1sb = []
        for oc in range(4):
            w_ = persist.tile([128, 128], bf16, tag=f"w1x1_{oc}")
            nc.gpsimd.dma_start(out=w_[:], in_=w1x1t_d[oc * 128:(oc + 1) * 128, :])
            w1x1sb.append(w_)
        w2d2sb = []
        for tap in range(9):
            w_ = persist.tile([128, 128], bf16, tag=f"w2d2_{tap}")
            nc.gpsimd.dma_start(out=w_[:], in_=w2d2t_d[tap])
            w2d2sb.append(w_)
        # wsmp resident in SBUF (bf16) as one tile per (blk, n) so matmuls
        # only wait on the slice they read; DMAs ride the scalar/gpsimd
        # HWDGE queues so they don't block the front's loads
        wsmpsb = {}
        for blk in range(NBLK):
            for n in range(NSMP):
                w_ = persist.tile([100, CB], bf16, tag=f"ws_{blk}_{n}",
                                  name=f"ws_{blk}_{n}")
                eng = nc.scalar if (n % 2 == 0) else nc.gpsimd
                eng.dma_start(out=w_[:],
                              in_=wsmp_d[:, n, blk * CB:(blk + 1) * CB])
                wsmpsb[blk, n] = w_
        w2d3sb = persist.tile([128, 2], f32)
        nc.sync.dma_start(out=w2d3sb[:], in_=w2d3t_d[:])

        # z1 maps: [128, TW trows, 102 dcols] per b  (d-pad cols 0 and 101)
        z1 = [z1_pool.tile([128, TW, 102], bf16, tag=f"z1b{b}", name=f"z1b{b}")
              for b in range(BATCH)]

        baseT = [persist.tile([100, 128], bf16, tag=f"baseT{b}", name=f"baseT{b}")
                 for b in range(BATCH)]

        # ---------------- front: conv1d stack + TEM + transposes
        with tc.tile_pool(name="front", bufs=1) as fr, \
             tc.tile_pool(name="front_ps", bufs=1, space="PSUM") as frps:
            # x -> sbuf [100, 4, 102] x4 chunks, t-padded
            x_sb = []
            for kc in range(4):
                t_ = fr.tile([100, BATCH, 102], f32r, tag=f"x{kc}")
                dma_zero(t_[:, :, 0:1])
                dma_zero(t_[:, :, 101:102])
                nc.sync.dma_start(
                    out=t_[:, :, 1:101],
                    in_=x_d[:, kc * 100:(kc + 1) * 100, :].rearrange("b c t -> c b t").bitcast(f32r))
                x_sb.append(t_)
            # conv weights
            wb1sb = {}
            for kc in range(4):
                for mc in range(2):
                    for tap in range(3):
                        w_ = fr.tile([100, 128], f32r, tag=f"wb1_{kc}_{mc}_{tap}")
                        nc.sync.dma_start(
                            out=w_[:],
                            in_=wb1t_d[tap, kc * 100:(kc + 1) * 100,
                                       mc * 128:(mc + 1) * 128].bitcast(f32r))
                        wb1sb[kc, mc, tap] = w_
            wb2sb = {}
            for kc in range(2):
                for tap in range(3):
                    w_ = fr.tile([128, 128], f32r, tag=f"wb2_{kc}_{tap}")
                    nc.sync.dma_start(
                        out=w_[:],
                        in_=wb2t_d[tap, kc * 128:(kc + 1) * 128, :].bitcast(f32r))
                    wb2sb[kc, tap] = w_
            wt1sb = {}
            for mc in range(2):
                for tap in range(3):
                    w_ = fr.tile([128, 128], f32, tag=f"wt1_{mc}_{tap}")
                    nc.sync.dma_start(
                        out=w_[:],
                        in_=wt1t_d[tap, :, mc * 128:(mc + 1) * 128])
                    wt1sb[mc, tap] = w_
            wt2sb = {}
            for kc in range(2):
                for tap in range(3):
                    w_ = fr.tile([128, 2], f32, tag=f"wt2_{kc}_{tap}")
                    nc.sync.dma_start(
                        out=w_[:],
                        in_=wt2t_d[tap, kc * 128:(kc + 1) * 128, :])
                    wt2sb[kc, tap] = w_

            # base1 = relu(conv1d(x))  [256 -> 2 chunks][100 t x 4 b]
            base1_sb = []
            for mc in range(2):
                ps = frps.tile([128, BATCH, 100], f32, tag="ps_b1")
                first = True
                for kc in range(4):
                    for tap in range(3):
                        nc.tensor.matmul(ps[:], wb1sb[kc, mc, tap][:],
                                         x_sb[kc][:, :, tap:tap + 100],
                                         start=first, stop=(kc == 3 and tap == 2))
                        first = False
                t_ = fr.tile([128, BATCH, 102], f32r, tag=f"base1_{mc}")
                dma_zero(t_[:, :, 0:1])
                dma_zero(t_[:, :, 101:102])
                nc.scalar.activation(t_[:, :, 1:101], ps[:], AF.Relu, bias=b1sb[mc][:])
                base1_sb.append(t_)

            # base = relu(conv1d(base1))  [128][4 b x 102]
            ps = frps.tile([128, BATCH, 100], f32, tag="ps_b2")
            first = True
            for kc in range(2):
                for tap in range(3):
                    nc.tensor.matmul(ps[:], wb2sb[kc, tap][:],
                                     base1_sb[kc][:, :, tap:tap + 100],
                                     start=first, stop=(kc == 1 and tap == 2))
                    first = False
            base_sb = fr.tile([128, BATCH, 102], f32)
            nc.vector.memset(base_sb[:], 0.0)
            nc.scalar.activation(base_sb[:, :, 1:101], ps[:], AF.Relu, bias=b2sb[0][:])

            # tem1 = relu(conv1d(base)) [2 chunks][4 x 102]
            tem1_sb = []
            for mc in range(2):
                ps = frps.tile([128, BATCH, 100], f32, tag="ps_t1")
                first = True
                for tap in range(3):
                    nc.tensor.matmul(ps[:], wt1sb[mc, tap][:],
                                     base_sb[:, :, tap:tap + 100],
                                     start=first, stop=(tap == 2))
                    first = False
                t_ = fr.tile([128, BATCH, 102], f32, tag=f"tem1_{mc}")
                nc.vector.memset(t_[:], 0.0)
                nc.scalar.activation(t_[:, :, 1:101], ps[:], AF.Relu, bias=bt1sb[mc][:])
                tem1_sb.append(t_)

            # tem_out = sigmoid(conv1d(tem1)) [2][4 x 100]
            ps = frps.tile([128, BATCH, 100], f32, tag="ps_t2")
            first = True
            for kc in range(2):
                for tap in range(3):
                    nc.tensor.matmul(ps[:2], wt2sb[kc, tap][:],
                                     tem1_sb[kc][:, :, tap:tap + 100],
                                     start=first, stop=(kc == 1 and tap == 2))
                    first = False
            temsb = fr.tile([2, BATCH, 100], f32)
            nc.scalar.activation(temsb[:], ps[:2], AF.Sigmoid, bias=bt2sb[:])
            for b in range(BATCH):
                nc.sync.dma_start(out=tem_d[b], in_=temsb[:, b, :])

            # baseT[b] = base[:, b, 1:101].T  -> [100 tt, 128 c]
            for b in range(BATCH):
                ps = frps.tile([128, 128], f32, tag="ps_tr")
                nc.tensor.transpose(ps[:100, :], base_sb[:, b, 1:101], ident[:])
                nc.vector.tensor_copy(baseT[b][:], ps[:100, :])

        # ---------------- phase A: sampling + conv3d + 1x1 over packed cols
        with tc.tile_pool(name="pem_sb", bufs=4) as pempool, \
             tc.tile_pool(name="y2", bufs=2) as y2pool, \
             tc.tile_pool(name="pa_ps", bufs=2, space="PSUM") as paps, \
             tc.tile_pool(name="y_ps", bufs=1, space="PSUM") as yps, \
             tc.tile_pool(name="z_ps", bufs=2, space="PSUM") as zps:
            for blk in range(NBLK):
                cs = blk * CB
                for b in range(BATCH):
                    y_ps = [yps.tile([128, CB], f32, tag=f"y{oc}", name=f"y_ps{oc}")
                            for oc in range(4)]
                    for n in range(NSMP):
                        pem_ps = paps.tile([128, CB], f32, tag="pem")
                        nc.tensor.matmul(pem_ps[:], baseT[b][:],
                                         wsmpsb[blk, n][:],
                                         start=True, stop=True)
                        pem_sb = pempool.tile([128, CB], bf16, tag="pem_sb")
                        if n % 2 == 0:
                            nc.vector.tensor_copy(pem_sb[:], pem_ps[:])
                        else:
                            nc.scalar.activation(pem_sb[:], pem_ps[:], AF.Copy)
                        for oc in range(4):
                            nc.tensor.matmul(
                                y_ps[oc][:],
                                w3rsb[n][:, oc * 128:(oc + 1) * 128],
                                pem_sb[:],
                                start=(n == 0), stop=(n == NSMP - 1))
                    y2 = []
                    for oc in range(4):
                        t_ = y2pool.tile([128, CB], bf16, tag=f"y2_{oc}")
                        nc.scalar.activation(t_[:], y_ps[oc][:], AF.Relu,
                                             bias=b3sb[oc][:])
                        y2.append(t_)
                    z_ps = zps.tile([128, CB], f32, tag="z")
                    for oc in range(4):
                        nc.tensor.matmul(z_ps[:], w1x1sb[oc][:], y2[oc][:],
                                         start=(oc == 0), stop=(oc == 3))
                    # cols are t-major (5 t-rows of 100 d) -> z1[b][:, 5blk:5blk+5, 1:101]
                    tr0 = cs // 100
                    nc.scalar.activation(
                        z1[b][:, tr0:tr0 + CB // 100, 1:101],
                        z_ps[:].rearrange("p (a c) -> p a c", a=CB // 100),
                        AF.Relu, bias=b2d1sb[0][:])

        # zero d-pad cols and mask out-of-range t rows
        for b in range(BATCH):
            nc.vector.memset(z1[b][:, :, 0:1], 0.0)
            nc.vector.memset(z1[b][:, :, 101:102], 0.0)
            for tr in range(TW):
                nc.vector.tensor_scalar_mul(z1[b][:, tr, 1:101],
                                            z1[b][:, tr, 1:101],
                                            tvalsb[:, tr:tr + 1])

        # ---------------- phase B: 3x3 conv + relu, 1x1 + sigmoid, DMA out
        DCH = [(0, 36), (36, 36), (72, 28)]
        with tc.tile_pool(name="z2", bufs=2) as z2pool, \
             tc.tile_pool(name="pb_ps", bufs=2, space="PSUM") as pbps, \
             tc.tile_pool(name="yo_ps", bufs=2, space="PSUM") as yops, \
             tc.tile_pool(name="yout", bufs=3) as yopool:
            for b in range(BATCH):
                for (d0, dn) in DCH:
                    # out cols ordered (t, d): rhs slices keep d contiguous
                    zz = pbps.tile([128, TC, dn], f32, tag=f"zz{dn}",
                                   name=f"zz{dn}")
                    for kh in (-1, 0, 1):        # d shift
                        for kw in (-1, 0, 1):    # t shift
                            tap = (kh + 1) * 3 + (kw + 1)
                            rhs = z1[b][:, 1 + kw:1 + kw + TC,
                                        d0 + 1 + kh:d0 + 1 + kh + dn]
                            nc.tensor.matmul(zz[:], w2d2sb[tap][:], rhs,
                                             start=(tap == 0), stop=(tap == 8))
                    z2 = z2pool.tile([128, TC, dn], f32, tag=f"z2{dn}",
                                     name=f"z2{dn}")
                    nc.scalar.activation(z2[:], zz[:], AF.Relu,
                                         bias=b2d2sb[0][:])
                    yo = yops.tile([128, TC, dn], f32, tag=f"yo{dn}",
                                   name=f"yo{dn}")
                    nc.tensor.matmul(yo[:2], w2d3sb[:], z2[:],
                                     start=True, stop=True)
                    yout = yopool.tile([2, TC, dn], f32, tag=f"yout{dn}",
                                       name=f"yout{dn}")
                    nc.scalar.activation(yout[:], yo[:2], AF.Sigmoid,
                                         bias=b2d3sb[:])
                    nc.sync.dma_start(out=y_d[b, :, :, d0:d0 + dn],
                                      in_=yout[:])

        z1_pool.release()
        persist.release()

    nc.compile()
    return nc


# ---------------------------------------------------------------- entry point
def _make_in_maps(inputs):
    import ml_dtypes
    wsmps, tvals, pr = _prep_host(inputs)
    shared = {
        "wb1t": pr["wb1t"], "b_base1": inputs["b_base1"],
        "wb2t": pr["wb2t"], "b_base2": inputs["b_base2"],
        "wt1t": pr["wt1t"], "b_tem1": inputs["b_tem1"],
        "wt2t": pr["wt2t"], "b_tem2": inputs["b_tem2"],
        "w3r": pr["w3r"], "b_c3d": inputs["b_c3d"],
        "w1x1t": pr["w1x1t"], "b_2d1": inputs["b_2d1"],
        "w2d2t": pr["w2d2t"], "b_2d2": inputs["b_2d2"],
        "w2d3t": pr["w2d3t"], "b_2d3": inputs["b_2d3"],
        "x": inputs["x"],
        "zeros": np.zeros(512, np.float32),
        "zerosh": np.zeros(512, ml_dtypes.bfloat16),
    }
    in_maps = []
    for r in range(NCORES):
        m = dict(shared)
        m["wsmp"] = wsmps[r]
        m["tval"] = tvals[r]
        in_maps.append(m)
    return in_maps


def kernel(**inputs):
    inputs = {k: np.asarray(v, dtype=np.float32) for k, v in inputs.items()}

    if "nc" not in _cache:
        _cache["nc"] = _build_program()
    nc = _cache["nc"]

    in_maps = _make_in_maps(inputs)

    from concourse.bass_utils import run_bass_kernel_spmd
    res = run_bass_kernel_spmd(nc, in_maps, list(range(NCORES)))
    _cache["last_res"] = res

    y = np.zeros((BATCH, 2, DPROP, T), np.float32)
    for r in range(NCORES):
        t0 = r * TC
        t1 = min(T, t0 + TC)
        yr = res.results[r]["y"].transpose(0, 1, 3, 2)   # -> [b, 2, d, t]
        y[:, :, :, t0:t1] = yr[:, :, :, : t1 - t0]
    tem = res.results[0]["tem"]
    return tem, y


# revision 24
# speedup vs baseline: 2.1883x; 1.2393x over previous
"""Trainium2 Bass kernel for the BMN-style nn module (nn_BMN_66683662238004).

Pipeline (per batch b):
  base = relu(conv1d(relu(conv1d(x))))            # [128, T]
  tem_out = sigmoid(conv1d(relu(conv1d(base))))   # [2, T]
  pem[c, n, d, t] = sum_tt base[c, tt] * Wsmp[tt, n, d, t]   (BM sampling)
  y1 = relu(conv3d(pem))   == per-(d,t) column: sum over n of W3_n[c,o] @ pem[c,n,(d,t)]
  z1 = relu(1x1(y1)); z2 = relu(3x3(z1)); y = sigmoid(1x1(z2))

Sharding: 8 cores, each owns a contiguous window of 13 t-columns (plus 1-col
halo each side for the 3x3 conv). Wsmp is precomputed on host (it is a
constant sparse interpolation matrix) and shipped pre-sliced per core.
All heavy matmuls run in fp32r (TF32-like) on the PE array.
"""

import os
import sys
import threading

import numpy as np

# ---------------------------------------------------------------- constants
T, NSMP, DPROP, EXPAND = 100, 32, 100, 0.5
FEAT, BATCH = 400, 4
NCORES = 8
TC = 13           # output t-columns per core (8*13 = 104 >= 100)
TW = TC + 2       # t-window incl halo
NCOLS = TW * 100  # packed phase-A columns per core (t-major, d-minor)
CB = 500          # phase-A column block (<=512 psum, >=256 for fp32r full rate)
NBLK = NCOLS // CB  # 3

_cache = {}


# ---------------------------------------------------------------- host math
def _smp_w4():
    """Faithful BMSampling weight, laid out [tt, n, t, d] float32."""
    ii = np.arange(T)                    # t (start index i)
    jj = np.arange(DPROP)                # d (duration index j)
    kk = np.arange(NSMP)
    J, I = np.meshgrid(jj, ii, indexing="ij")        # [d, t]
    valid_ij = J < np.minimum(T - 1 - I, DPROP)      # j < min(T-1-i, D); i<=T-2 implied
    length = (J + 1 - I).astype(np.float64)
    xmin_ext = I - length * EXPAND
    bin_size = (length + 2 * EXPAND * length) / (NSMP - 1)
    xp = xmin_ext[None] + kk[:, None, None] * bin_size[None]   # [n, d, t]
    ok = valid_ij[None] & (xp >= 0) & (xp <= T - 1)
    left = np.floor(xp).astype(np.int64)
    right = np.ceil(xp).astype(np.int64)
    wl = 1.0 - (xp - left)
    wr = 1.0 - (right - xp)
    w = np.zeros((T, NSMP, T, DPROP), np.float32)    # [tt, n, t, d]
    n_i, d_i, t_i = np.nonzero(ok)
    np.add.at(w, (left[ok], n_i, t_i, d_i), wl[ok])
    np.add.at(w, (right[ok], n_i, t_i, d_i), wr[ok])
    return w


def _prep_host(inputs):
    """Host-side constant prep: Wsmp slices + transposed weights."""
    import ml_dtypes
    bf16 = ml_dtypes.bfloat16
    w4 = _smp_w4()                                   # [tt, n, t, d]
    wsmps, tvals = [], []
    for r in range(NCORES):
        t0 = r * TC - 1
        sl = np.zeros((T, NSMP, TW, DPROP), np.float32)
        lo, hi = max(0, t0), min(T, t0 + TW)
        sl[:, :, lo - t0 : hi - t0, :] = w4[:, :, lo:hi, :]
        wsmps.append(np.ascontiguousarray(sl.reshape(T, NSMP, NCOLS).astype(bf16)))
        tv = np.zeros(TW, np.float32)
        tv[lo - t0 : hi - t0] = 1.0
        tvals.append(tv)

    pr = {
        "wb1t": np.ascontiguousarray(inputs["w_base1"].transpose(2, 1, 0)),  # [3,400,256]
        "wb2t": np.ascontiguousarray(inputs["w_base2"].transpose(2, 1, 0)),  # [3,256,128]
        "wt1t": np.ascontiguousarray(inputs["w_tem1"].transpose(2, 1, 0)),   # [3,128,256]
        "wt2t": np.ascontiguousarray(inputs["w_tem2"].transpose(2, 1, 0)),   # [3,256,2]
        "w3r": np.ascontiguousarray(inputs["w_c3d"].transpose(2, 1, 0)),     # [32,128,512]
        "w1x1t": np.ascontiguousarray(
            inputs["w_2d1"].reshape(128, 512).transpose(1, 0)),              # [512,128]
        "w2d2t": np.ascontiguousarray(
            inputs["w_2d2"].transpose(2, 3, 1, 0).reshape(9, 128, 128)),     # [kh*3+kw,c,o]
        "w2d3t": np.ascontiguousarray(inputs["w_2d3"].reshape(2, 128).transpose(1, 0)),
    }
    for k in ("w3r", "w1x1t", "w2d2t"):
        pr[k] = pr[k].astype(bf16)
    return wsmps, tvals, pr


# ---------------------------------------------------------------- device build
def _build_program():
    import concourse.bass as bass
    import concourse.tile as tile
    from concourse import bacc, mybir
    from concourse.masks import make_identity

    f32 = mybir.dt.float32
    f32r = mybir.dt.float32r
    bf16 = mybir.dt.bfloat16
    AF = mybir.ActivationFunctionType

    nc = bacc.Bacc("TRN2", target_bir_lowering=False, debug=False,
                   num_devices=NCORES)

    def din(name, shape):
        return nc.dram_tensor(name, shape, f32, kind="ExternalInput").ap()

    x_d = din("x", [BATCH, FEAT, T])
    zeros_d = din("zeros", [512])
    zerosh_d = nc.dram_tensor("zerosh", [512], mybir.dt.bfloat16,
                              kind="ExternalInput").ap()
    wsmp_d = nc.dram_tensor("wsmp", [T, NSMP, NCOLS], mybir.dt.bfloat16,
                            kind="ExternalInput").ap()
    tval_d = din("tval", [TW])
    wb1t_d = din("wb1t", [3, 400, 256])
    b1_d = din("b_base1", [256])
    wb2t_d = din("wb2t", [3, 256, 128])
    b2_d = din("b_base2", [128])
    wt1t_d = din("wt1t", [3, 128, 256])
    bt1_d = din("b_tem1", [256])
    wt2t_d = din("wt2t", [3, 256, 2])
    bt2_d = din("b_tem2", [2])
    w3r_d = nc.dram_tensor("w3r", [NSMP, 128, 512], mybir.dt.bfloat16,
                           kind="ExternalInput").ap()
    b3_d = din("b_c3d", [512])
    w1x1t_d = nc.dram_tensor("w1x1t", [512, 128], mybir.dt.bfloat16,
                             kind="ExternalInput").ap()
    b2d1_d = din("b_2d1", [128])
    w2d2t_d = nc.dram_tensor("w2d2t", [9, 128, 128], mybir.dt.bfloat16,
                             kind="ExternalInput").ap()
    b2d2_d = din("b_2d2", [128])
    w2d3t_d = din("w2d3t", [128, 2])
    b2d3_d = din("b_2d3", [2])

    # stored (t, d) on device; host transposes to (d, t)
    y_d = nc.dram_tensor("y", [BATCH, 2, TC, DPROP], f32, kind="ExternalOutput").ap()
    tem_d = nc.dram_tensor("tem", [BATCH, 2, T], f32, kind="ExternalOutput").ap()

    with tile.TileContext(nc) as tc:
        # ---------------- persistent pools
        persist = tc.alloc_tile_pool(name="persist", bufs=1)
        z1_pool = tc.alloc_tile_pool(name="z1", bufs=1)

        ident = persist.tile([128, 128], f32)
        make_identity(nc, ident)

        # biases as [p,1] tiles
        def bias_tiles(src, n_chunks, tag):
            ts = []
            for i in range(n_chunks):
                t_ = persist.tile([128, 1], f32, tag=f"{tag}{i}")
                nc.gpsimd.dma_start(out=t_[:, 0], in_=src[i * 128:(i + 1) * 128])
                ts.append(t_)
            return ts

        b1sb = bias_tiles(b1_d, 2, "b1")
        b2sb = bias_tiles(b2_d, 1, "b2")
        bt1sb = bias_tiles(bt1_d, 2, "bt1")
        b3sb = bias_tiles(b3_d, 4, "b3")
        b2d1sb = bias_tiles(b2d1_d, 1, "b2d1")
        b2d2sb = bias_tiles(b2d2_d, 1, "b2d2")
        bt2sb = persist.tile([2, 1], f32)
        nc.sync.dma_start(out=bt2sb[:, 0], in_=bt2_d[:])
        b2d3sb = persist.tile([2, 1], f32)
        nc.sync.dma_start(out=b2d3sb[:, 0], in_=b2d3_d[:])

        tvalsb = persist.tile([128, TW], f32)
        nc.sync.dma_start(
            out=tvalsb[:],
            in_=bass.AP(tensor=tval_d.tensor, offset=tval_d.offset,
                        ap=[[0, 128], *tval_d.ap]))

        def dma_zero(out_ap):
            """Zero-fill an f32r/f32 tile region via DMA from the zeros input
            (memset ISA does not support float32r)."""
            dims = out_ap.shape
            if out_ap.dtype == bf16:
                srct = zerosh_d
            elif out_ap.dtype == f32:
                srct = zeros_d
            else:
                srct = zeros_d.bitcast(out_ap.dtype)
            ap = [[0, dims[0]]] + [[0, d] for d in dims[1:-1]] + [[1, dims[-1]]]
            nc.sync.dma_start(
                out=out_ap,
                in_=bass.AP(tensor=srct.tensor, offset=srct.offset, ap=ap))

        # w3r resident [32][128, 512], w1x1 [4][128,128], w2d2 taps, w2d3
        w3rsb = []
        for n in range(NSMP):
            w_ = persist.tile([128, 512], bf16, tag=f"w3r{n}")
            nc.gpsimd.dma_start(out=w_[:], in_=w3r_d[n])
            w3rsb.append(w_)
        w1x1sb = []
        for oc in range(4):
            w_ = persist.tile([128, 128], bf16, tag=f"w1x1_{oc}")
            nc.gpsimd.dma_start(out=w_[:], in_=w1x1t_d[oc * 128:(oc + 1) * 128, :])
            w1x1sb.append(w_)
        w2d2sb = []
        for tap in range(9):
            w_ = persist.tile([128, 128], bf16, tag=f"w2d2_{tap}")
            nc.gpsimd.dma_start(out=w_[:], in_=w2d2t_d[tap])
            w2d2sb.append(w_)
        # wsmp resident in SBUF (bf16) as one tile per (blk, n) so matmuls
        # only wait on the slice they read; DMAs ride the scalar/gpsimd
        # HWDGE queues so they don't block the front's loads
        wsmpsb = {}
        for blk in range(NBLK):
            for n in range(NSMP):
                w_ = persist.tile([100, CB], bf16, tag=f"ws_{blk}_{n}",
                                  name=f"ws_{blk}_{n}")
                eng = nc.scalar if (n % 2 == 0) else nc.gpsimd
                eng.dma_start(out=w_[:],
                              in_=wsmp_d[:, n, blk * CB:(blk + 1) * CB])
                wsmpsb[blk, n] = w_
        w2d3sb = persist.tile([128, 2], f32)
        nc.sync.dma_start(out=w2d3sb[:], in_=w2d3t_d[:])

        # z1 maps: [128, TW trows, 102 dcols] per b  (d-pad cols 0 and 101)
        z1 = [z1_pool.tile([128, TW, 102], bf16, tag=f"z1b{b}", name=f"z1b{b}")
              for b in range(BATCH)]

        baseT = [persist.tile([100, 128], bf16, tag=f"baseT{b}", name=f"baseT{b}")
                 for b in range(BATCH)]

        # ---------------- front: conv1d stack + TEM + transposes
        with tc.tile_pool(name="front", bufs=1) as fr, \
             tc.tile_pool(name="front_ps", bufs=1, space="PSUM") as frps:
            # x -> sbuf [100, 4, 102] x4 chunks, t-padded
            x_sb = []
            for kc in range(4):
                t_ = fr.tile([100, BATCH, 102], f32r, tag=f"x{kc}")
                dma_zero(t_[:, :, 0:1])
                dma_zero(t_[:, :, 101:102])
                nc.sync.dma_start(
                    out=t_[:, :, 1:101],
                    in_=x_d[:, kc * 100:(kc + 1) * 100, :].rearrange("b c t -> c b t").bitcast(f32r))
                x_sb.append(t_)
            # conv weights
            wb1sb = {}
            for kc in range(4):
                for mc in range(2):
                    for tap in range(3):
                        w_ = fr.tile([100, 128], f32r, tag=f"wb1_{kc}_{mc}_{tap}")
                        nc.sync.dma_start(
                            out=w_[:],
                            in_=wb1t_d[tap, kc * 100:(kc + 1) * 100,
                                       mc * 128:(mc + 1) * 128].bitcast(f32r))
                        wb1sb[kc, mc, tap] = w_
            wb2sb = {}
            for kc in range(2):
                for tap in range(3):
                    w_ = fr.tile([128, 128], f32r, tag=f"wb2_{kc}_{tap}")
                    nc.sync.dma_start(
                        out=w_[:],
                        in_=wb2t_d[tap, kc * 128:(kc + 1) * 128, :].bitcast(f32r))
                    wb2sb[kc, tap] = w_
            wt1sb = {}
            for mc in range(2):
                for tap in range(3):
                    w_ = fr.tile([128, 128], f32, tag=f"wt1_{mc}_{tap}")
                    nc.sync.dma_start(
                        out=w_[:],
                        in_=wt1t_d[tap, :, mc * 128:(mc + 1) * 128])
                    wt1sb[mc, tap] = w_
            wt2sb = {}
            for kc in range(2):
                for tap in range(3):
                    w_ = fr.tile([128, 2], f32, tag=f"wt2_{kc}_{tap}")
                    nc.sync.dma_start(
                        out=w_[:],
                        in_=wt2t_d[tap, kc * 128:(kc + 1) * 128, :])
                    wt2sb[kc, tap] = w_

            # base1 = relu(conv1d(x))  [256 -> 2 chunks][100 t x 4 b]
            base1_sb = []
            for mc in range(2):
                ps = frps.tile([128, BATCH, 100], f32, tag="ps_b1")
                first = True
                for kc in range(4):
                    for tap in range(3):
                        nc.tensor.matmul(ps[:], wb1sb[kc, mc, tap][:],
                                         x_sb[kc][:, :, tap:tap + 100],
                                         start=first, stop=(kc == 3 and tap == 2))
                        first = False
                t_ = fr.tile([128, BATCH, 102], f32r, tag=f"base1_{mc}")
                dma_zero(t_[:, :, 0:1])
                dma_zero(t_[:, :, 101:102])
                nc.scalar.activation(t_[:, :, 1:101], ps[:], AF.Relu, bias=b1sb[mc][:])
                base1_sb.append(t_)

            # base = relu(conv1d(base1))  [128][4 b x 102]
            ps = frps.tile([128, BATCH, 100], f32, tag="ps_b2")
            first = True
            for kc in range(2):
                for tap in range(3):
                    nc.tensor.matmul(ps[:], wb2sb[kc, tap][:],
                                     base1_sb[kc][:, :, tap:tap + 100],
                                     start=first, stop=(kc == 1 and tap == 2))
                    first = False
            base_sb = fr.tile([128, BATCH, 102], f32)
            nc.vector.memset(base_sb[:], 0.0)
            nc.scalar.activation(base_sb[:, :, 1:101], ps[:], AF.Relu, bias=b2sb[0][:])

            # tem1 = relu(conv1d(base)) [2 chunks][4 x 102]
            tem1_sb = []
            for mc in range(2):
                ps = frps.tile([128, BATCH, 100], f32, tag="ps_t1")
                first = True
                for tap in range(3):
                    nc.tensor.matmul(ps[:], wt1sb[mc, tap][:],
                                     base_sb[:, :, tap:tap + 100],
                                     start=first, stop=(tap == 2))
                    first = False
                t_ = fr.tile([128, BATCH, 102], f32, tag=f"tem1_{mc}")
                nc.vector.memset(t_[:], 0.0)
                nc.scalar.activation(t_[:, :, 1:101], ps[:], AF.Relu, bias=bt1sb[mc][:])
                tem1_sb.append(t_)

            # tem_out = sigmoid(conv1d(tem1)) [2][4 x 100]
            ps = frps.tile([128, BATCH, 100], f32, tag="ps_t2")
            first = True
            for kc in range(2):
                for tap in range(3):
                    nc.tensor.matmul(ps[:2], wt2sb[kc, tap][:],
                                     tem1_sb[kc][:, :, tap:tap + 100],
                                     start=first, stop=(kc == 1 and tap == 2))
                    first = False
            temsb = fr.tile([2, BATCH, 100], f32)
            nc.scalar.activation(temsb[:], ps[:2], AF.Sigmoid, bias=bt2sb[:])
            for b in range(BATCH):
                nc.sync.dma_start(out=tem_d[b], in_=temsb[:, b, :])

            # baseT[b] = base[:, b, 1:101].T  -> [100 tt, 128 c]
            for b in range(BATCH):
                ps = frps.tile([128, 128], f32, tag="ps_tr")
                nc.tensor.transpose(ps[:100, :], base_sb[:, b, 1:101], ident[:])
                nc.vector.tensor_copy(baseT[b][:], ps[:100, :])

        # ---------------- phase A: sampling + conv3d + 1x1 over packed cols
        with tc.tile_pool(name="pem_sb", bufs=4) as pempool, \
             tc.tile_pool(name="y2", bufs=2) as y2pool, \
             tc.tile_pool(name="pa_ps", bufs=3, space="PSUM") as paps, \
             tc.tile_pool(name="y_ps", bufs=1, space="PSUM") as yps, \
             tc.tile_pool(name="z_ps", bufs=1, space="PSUM") as zps:
            for blk in range(NBLK):
                cs = blk * CB
                for b in range(BATCH):
                    y_ps = [yps.tile([128, CB], f32, tag=f"y{oc}", name=f"y_ps{oc}")
                            for oc in range(4)]
                    # software-pipeline: pem runs 2 iterations ahead of the
                    # conv3d consumers so the PSUM->SBUF copy is off PE's
                    # critical path
                    pem_sbs = {}

                    def do_pem(n):
                        pem_ps = paps.tile([128, CB], f32, tag="pem",
                                           name="pem_ps")
                        nc.tensor.matmul(pem_ps[:], baseT[b][:],
                                         wsmpsb[blk, n][:],
                                         start=True, stop=True)
                        pem_sb = pempool.tile([128, CB], bf16, tag="pem_sb",
                                              name="pem_sb")
                        if n % 2 == 0:
                            nc.vector.tensor_copy(pem_sb[:], pem_ps[:])
                        else:
                            nc.scalar.activation(pem_sb[:], pem_ps[:], AF.Copy)
                        pem_sbs[n] = pem_sb

                    do_pem(0)
                    do_pem(1)
                    for n in range(NSMP):
                        if n + 2 < NSMP:
                            do_pem(n + 2)
                        pem_sb = pem_sbs.pop(n)
                        for oc in range(4):
                            nc.tensor.matmul(
                                y_ps[oc][:],
                                w3rsb[n][:, oc * 128:(oc + 1) * 128],
                                pem_sb[:],
                                start=(n == 0), stop=(n == NSMP - 1))
                    y2 = []
                    for oc in range(4):
                        t_ = y2pool.tile([128, CB], bf16, tag=f"y2_{oc}")
                        nc.scalar.activation(t_[:], y_ps[oc][:], AF.Relu,
                                             bias=b3sb[oc][:])
                        y2.append(t_)
                    z_ps = zps.tile([128, CB], f32, tag="z")
                    for oc in range(4):
                        nc.tensor.matmul(z_ps[:], w1x1sb[oc][:], y2[oc][:],
                                         start=(oc == 0), stop=(oc == 3))
                    # cols are t-major (5 t-rows of 100 d) -> z1[b][:, 5blk:5blk+5, 1:101]
                    tr0 = cs // 100
                    nc.scalar.activation(
                        z1[b][:, tr0:tr0 + CB // 100, 1:101],
                        z_ps[:].rearrange("p (a c) -> p a c", a=CB // 100),
                        AF.Relu, bias=b2d1sb[0][:])

        # zero d-pad cols and mask out-of-range t rows
        for b in range(BATCH):
            nc.vector.memset(z1[b][:, :, 0:1], 0.0)
            nc.vector.memset(z1[b][:, :, 101:102], 0.0)
            for tr in range(TW):
                nc.vector.tensor_scalar_mul(z1[b][:, tr, 1:101],
                                            z1[b][:, tr, 1:101],
                                            tvalsb[:, tr:tr + 1])

        # ---------------- phase B: 3x3 conv + relu, 1x1 + sigmoid, DMA out
        DCH = [(0, 36), (36, 36), (72, 28)]
        with tc.tile_pool(name="z2", bufs=2) as z2pool, \
             tc.tile_pool(name="pb_ps", bufs=2, space="PSUM") as pbps, \
             tc.tile_pool(name="yo_ps", bufs=2, space="PSUM") as yops, \
             tc.tile_pool(name="yout", bufs=3) as yopool:
            for b in range(BATCH):
                for (d0, dn) in DCH:
                    # out cols ordered (t, d): rhs slices keep d contiguous
                    zz = pbps.tile([128, TC, dn], f32, tag=f"zz{dn}",
                                   name=f"zz{dn}")
                    for kh in (-1, 0, 1):        # d shift
                        for kw in (-1, 0, 1):    # t shift
                            tap = (kh + 1) * 3 + (kw + 1)
                            rhs = z1[b][:, 1 + kw:1 + kw + TC,
                                        d0 + 1 + kh:d0 + 1 + kh + dn]
                            nc.tensor.matmul(zz[:], w2d2sb[tap][:], rhs,
                                             start=(tap == 0), stop=(tap == 8))
                    z2 = z2pool.tile([128, TC, dn], f32, tag=f"z2{dn}",
                                     name=f"z2{dn}")
                    nc.scalar.activation(z2[:], zz[:], AF.Relu,
                                         bias=b2d2sb[0][:])
                    yo = yops.tile([128, TC, dn], f32, tag=f"yo{dn}",
                                   name=f"yo{dn}")
                    nc.tensor.matmul(yo[:2], w2d3sb[:], z2[:],
                                     start=True, stop=True)
                    yout = yopool.tile([2, TC, dn], f32, tag=f"yout{dn}",
                                       name=f"yout{dn}")
                    nc.scalar.activation(yout[:], yo[:2], AF.Sigmoid,
                                         bias=b2d3sb[:])
                    nc.sync.dma_start(out=y_d[b, :, :, d0:d0 + dn],
                                      in_=yout[:])

        z1_pool.release()
        persist.release()

    nc.compile()
    return nc


# ---------------------------------------------------------------- entry point
def _make_in_maps(inputs):
    import ml_dtypes
    wsmps, tvals, pr = _prep_host(inputs)
    shared = {
        "wb1t": pr["wb1t"], "b_base1": inputs["b_base1"],
        "wb2t": pr["wb2t"], "b_base2": inputs["b_base2"],
        "wt1t": pr["wt1t"], "b_tem1": inputs["b_tem1"],
        "wt2t": pr["wt2t"], "b_tem2": inputs["b_tem2"],
        "w3r": pr["w3r"], "b_c3d": inputs["b_c3d"],
        "w1x1t": pr["w1x1t"], "b_2d1": inputs["b_2d1"],
        "w2d2t": pr["w2d2t"], "b_2d2": inputs["b_2d2"],
        "w2d3t": pr["w2d3t"], "b_2d3": inputs["b_2d3"],
        "x": inputs["x"],
        "zeros": np.zeros(512, np.float32),
        "zerosh": np.zeros(512, ml_dtypes.bfloat16),
    }
    in_maps = []
    for r in range(NCORES):
        m = dict(shared)
        m["wsmp"] = wsmps[r]
        m["tval"] = tvals[r]
        in_maps.append(m)
    return in_maps


def kernel(**inputs):
    inputs = {k: np.asarray(v, dtype=np.float32) for k, v in inputs.items()}

    if "nc" not in _cache:
        _cache["nc"] = _build_program()
    nc = _cache["nc"]

    in_maps = _make_in_maps(inputs)

    from concourse.bass_utils import run_bass_kernel_spmd
    res = run_bass_kernel_spmd(nc, in_maps, list(range(NCORES)))
    _cache["last_res"] = res

    y = np.zeros((BATCH, 2, DPROP, T), np.float32)
    for r in range(NCORES):
        t0 = r * TC
        t1 = min(T, t0 + TC)
        yr = res.results[r]["y"].transpose(0, 1, 3, 2)   # -> [b, 2, d, t]
        y[:, :, :, t0:t1] = yr[:, :, :, : t1 - t0]
    tem = res.results[0]["tem"]
    return tem, y


# revision 25
# speedup vs baseline: 2.2238x; 1.0162x over previous
"""Trainium2 Bass kernel for the BMN-style nn module (nn_BMN_66683662238004).

Pipeline (per batch b):
  base = relu(conv1d(relu(conv1d(x))))            # [128, T]
  tem_out = sigmoid(conv1d(relu(conv1d(base))))   # [2, T]
  pem[c, n, d, t] = sum_tt base[c, tt] * Wsmp[tt, n, d, t]   (BM sampling)
  y1 = relu(conv3d(pem))   == per-(d,t) column: sum over n of W3_n[c,o] @ pem[c,n,(d,t)]
  z1 = relu(1x1(y1)); z2 = relu(3x3(z1)); y = sigmoid(1x1(z2))

Sharding: 8 cores, each owns a contiguous window of 13 t-columns (plus 1-col
halo each side for the 3x3 conv). Wsmp is precomputed on host (it is a
constant sparse interpolation matrix) and shipped pre-sliced per core.
All heavy matmuls run in fp32r (TF32-like) on the PE array.
"""

import os
import sys
import threading

import numpy as np

# ---------------------------------------------------------------- constants
T, NSMP, DPROP, EXPAND = 100, 32, 100, 0.5
FEAT, BATCH = 400, 4
NCORES = 8
TC = 13           # output t-columns per core (8*13 = 104 >= 100)
TW = TC + 2       # t-window incl halo
NCOLS = TW * 100  # packed phase-A columns per core (t-major, d-minor)
CB = 500          # phase-A column block (<=512 psum, >=256 for fp32r full rate)
NBLK = NCOLS // CB  # 3

_cache = {}


# ---------------------------------------------------------------- host math
def _smp_w4():
    """Faithful BMSampling weight, laid out [tt, n, t, d] float32."""
    ii = np.arange(T)                    # t (start index i)
    jj = np.arange(DPROP)                # d (duration index j)
    kk = np.arange(NSMP)
    J, I = np.meshgrid(jj, ii, indexing="ij")        # [d, t]
    valid_ij = J < np.minimum(T - 1 - I, DPROP)      # j < min(T-1-i, D); i<=T-2 implied
    length = (J + 1 - I).astype(np.float64)
    xmin_ext = I - length * EXPAND
    bin_size = (length + 2 * EXPAND * length) / (NSMP - 1)
    xp = xmin_ext[None] + kk[:, None, None] * bin_size[None]   # [n, d, t]
    ok = valid_ij[None] & (xp >= 0) & (xp <= T - 1)
    left = np.floor(xp).astype(np.int64)
    right = np.ceil(xp).astype(np.int64)
    wl = 1.0 - (xp - left)
    wr = 1.0 - (right - xp)
    w = np.zeros((T, NSMP, T, DPROP), np.float32)    # [tt, n, t, d]
    n_i, d_i, t_i = np.nonzero(ok)
    np.add.at(w, (left[ok], n_i, t_i, d_i), wl[ok])
    np.add.at(w, (right[ok], n_i, t_i, d_i), wr[ok])
    return w


def _prep_host(inputs):
    """Host-side constant prep: Wsmp slices + transposed weights."""
    import ml_dtypes
    bf16 = ml_dtypes.bfloat16
    w4 = _smp_w4()                                   # [tt, n, t, d]
    wsmps, tvals = [], []
    for r in range(NCORES):
        t0 = r * TC - 1
        sl = np.zeros((T, NSMP, TW, DPROP), np.float32)
        lo, hi = max(0, t0), min(T, t0 + TW)
        sl[:, :, lo - t0 : hi - t0, :] = w4[:, :, lo:hi, :]
        wsmps.append(np.ascontiguousarray(sl.reshape(T, NSMP, NCOLS).astype(bf16)))
        tv = np.zeros(TW, np.float32)
        tv[lo - t0 : hi - t0] = 1.0
        tvals.append(tv)

    pr = {
        "wb1t": np.ascontiguousarray(inputs["w_base1"].transpose(2, 1, 0)),  # [3,400,256]
        "wb2t": np.ascontiguousarray(inputs["w_base2"].transpose(2, 1, 0)),  # [3,256,128]
        "wt1t": np.ascontiguousarray(inputs["w_tem1"].transpose(2, 1, 0)),   # [3,128,256]
        "wt2t": np.ascontiguousarray(inputs["w_tem2"].transpose(2, 1, 0)),   # [3,256,2]
        "w3r": np.ascontiguousarray(inputs["w_c3d"].transpose(2, 1, 0)),     # [32,128,512]
        "w1x1t": np.ascontiguousarray(
            inputs["w_2d1"].reshape(128, 512).transpose(1, 0)),              # [512,128]
        "w2d2t": np.ascontiguousarray(
            inputs["w_2d2"].transpose(2, 3, 1, 0).reshape(9, 128, 128)),     # [kh*3+kw,c,o]
        "w2d3t": np.ascontiguousarray(inputs["w_2d3"].reshape(2, 128).transpose(1, 0)),
    }
    for k in ("w3r", "w1x1t", "w2d2t"):
        pr[k] = pr[k].astype(bf16)
    return wsmps, tvals, pr


# ---------------------------------------------------------------- device build
def _build_program():
    import concourse.bass as bass
    import concourse.tile as tile
    from concourse import bacc, mybir
    from concourse.masks import make_identity

    f32 = mybir.dt.float32
    f32r = mybir.dt.float32r
    bf16 = mybir.dt.bfloat16
    AF = mybir.ActivationFunctionType

    nc = bacc.Bacc("TRN2", target_bir_lowering=False, debug=False,
                   num_devices=NCORES)

    def din(name, shape):
        return nc.dram_tensor(name, shape, f32, kind="ExternalInput").ap()

    x_d = din("x", [BATCH, FEAT, T])
    zeros_d = din("zeros", [512])
    zerosh_d = nc.dram_tensor("zerosh", [512], mybir.dt.bfloat16,
                              kind="ExternalInput").ap()
    wsmp_d = nc.dram_tensor("wsmp", [T, NSMP, NCOLS], mybir.dt.bfloat16,
                            kind="ExternalInput").ap()
    tval_d = din("tval", [TW])
    wb1t_d = din("wb1t", [3, 400, 256])
    b1_d = din("b_base1", [256])
    wb2t_d = din("wb2t", [3, 256, 128])
    b2_d = din("b_base2", [128])
    wt1t_d = din("wt1t", [3, 128, 256])
    bt1_d = din("b_tem1", [256])
    wt2t_d = din("wt2t", [3, 256, 2])
    bt2_d = din("b_tem2", [2])
    w3r_d = nc.dram_tensor("w3r", [NSMP, 128, 512], mybir.dt.bfloat16,
                           kind="ExternalInput").ap()
    b3_d = din("b_c3d", [512])
    w1x1t_d = nc.dram_tensor("w1x1t", [512, 128], mybir.dt.bfloat16,
                             kind="ExternalInput").ap()
    b2d1_d = din("b_2d1", [128])
    w2d2t_d = nc.dram_tensor("w2d2t", [9, 128, 128], mybir.dt.bfloat16,
                             kind="ExternalInput").ap()
    b2d2_d = din("b_2d2", [128])
    w2d3t_d = din("w2d3t", [128, 2])
    b2d3_d = din("b_2d3", [2])

    # stored (t, d) on device; host transposes to (d, t)
    y_d = nc.dram_tensor("y", [BATCH, 2, TC, DPROP], f32, kind="ExternalOutput").ap()
    tem_d = nc.dram_tensor("tem", [BATCH, 2, T], f32, kind="ExternalOutput").ap()

    with tile.TileContext(nc) as tc:
        # ---------------- persistent pools
        persist = tc.alloc_tile_pool(name="persist", bufs=1)
        z1_pool = tc.alloc_tile_pool(name="z1", bufs=1)

        ident = persist.tile([128, 128], f32)
        make_identity(nc, ident)

        # biases as [p,1] tiles
        def bias_tiles(src, n_chunks, tag):
            ts = []
            for i in range(n_chunks):
                t_ = persist.tile([128, 1], f32, tag=f"{tag}{i}")
                nc.gpsimd.dma_start(out=t_[:, 0], in_=src[i * 128:(i + 1) * 128])
                ts.append(t_)
            return ts

        b1sb = bias_tiles(b1_d, 2, "b1")
        b2sb = bias_tiles(b2_d, 1, "b2")
        bt1sb = bias_tiles(bt1_d, 2, "bt1")
        b3sb = bias_tiles(b3_d, 4, "b3")
        b2d1sb = bias_tiles(b2d1_d, 1, "b2d1")
        b2d2sb = bias_tiles(b2d2_d, 1, "b2d2")
        bt2sb = persist.tile([2, 1], f32)
        nc.sync.dma_start(out=bt2sb[:, 0], in_=bt2_d[:])
        b2d3sb = persist.tile([2, 1], f32)
        nc.sync.dma_start(out=b2d3sb[:, 0], in_=b2d3_d[:])

        tvalsb = persist.tile([128, TW], f32)
        nc.sync.dma_start(
            out=tvalsb[:],
            in_=bass.AP(tensor=tval_d.tensor, offset=tval_d.offset,
                        ap=[[0, 128], *tval_d.ap]))

        def dma_zero(out_ap):
            """Zero-fill an f32r/f32 tile region via DMA from the zeros input
            (memset ISA does not support float32r)."""
            dims = out_ap.shape
            if out_ap.dtype == bf16:
                srct = zerosh_d
            elif out_ap.dtype == f32:
                srct = zeros_d
            else:
                srct = zeros_d.bitcast(out_ap.dtype)
            ap = [[0, dims[0]]] + [[0, d] for d in dims[1:-1]] + [[1, dims[-1]]]
            nc.sync.dma_start(
                out=out_ap,
                in_=bass.AP(tensor=srct.tensor, offset=srct.offset, ap=ap))

        # w3r resident [32][128, 512], w1x1 [4][128,128], w2d2 taps, w2d3
        w3rsb = []
        for n in range(NSMP):
            w_ = persist.tile([128, 512], bf16, tag=f"w3r{n}")
            w3rsb.append(w_)
        w1x1sb = []
        for oc in range(4):
            w_ = persist.tile([128, 128], bf16, tag=f"w1x1_{oc}")
            nc.gpsimd.dma_start(out=w_[:], in_=w1x1t_d[oc * 128:(oc + 1) * 128, :])
            w1x1sb.append(w_)
        w2d2sb = []
        for tap in range(9):
            w_ = persist.tile([128, 128], bf16, tag=f"w2d2_{tap}")
            nc.gpsimd.dma_start(out=w_[:], in_=w2d2t_d[tap])
            w2d2sb.append(w_)
        # wsmp resident in SBUF (bf16) as one tile per (blk, n) so matmuls
        # only wait on the slice they read; DMAs ride the scalar/gpsimd
        # HWDGE queues so they don't block the front's loads
        wsmpsb = {}
        for blk in range(NBLK):
            for n in range(NSMP):
                w_ = persist.tile([100, CB], bf16, tag=f"ws_{blk}_{n}",
                                  name=f"ws_{blk}_{n}")
                eng = nc.scalar if (n % 2 == 0) else nc.gpsimd
                eng.dma_start(out=w_[:],
                              in_=wsmp_d[:, n, blk * CB:(blk + 1) * CB])
                wsmpsb[blk, n] = w_
        w2d3sb = persist.tile([128, 2], f32)
        nc.sync.dma_start(out=w2d3sb[:], in_=w2d3t_d[:])

        # z1 maps: [128, TW trows, 102 dcols] per b  (d-pad cols 0 and 101)
        z1 = [z1_pool.tile([128, TW, 102], bf16, tag=f"z1b{b}", name=f"z1b{b}")
              for b in range(BATCH)]

        baseT = [persist.tile([100, 128], bf16, tag=f"baseT{b}", name=f"baseT{b}")
                 for b in range(BATCH)]

        # ---------------- front: conv1d stack + TEM + transposes
        with tc.tile_pool(name="front", bufs=1) as fr, \
             tc.tile_pool(name="front_ps", bufs=1, space="PSUM") as frps:
            # x -> sbuf [100, 4, 102] x4 chunks, t-padded
            x_sb = []
            for kc in range(4):
                t_ = fr.tile([100, BATCH, 102], f32r, tag=f"x{kc}")
                dma_zero(t_[:, :, 0:1])
                dma_zero(t_[:, :, 101:102])
                nc.sync.dma_start(
                    out=t_[:, :, 1:101],
                    in_=x_d[:, kc * 100:(kc + 1) * 100, :].rearrange("b c t -> c b t").bitcast(f32r))
                x_sb.append(t_)
            # conv weights
            wb1sb = {}
            for kc in range(4):
                for mc in range(2):
                    for tap in range(3):
                        w_ = fr.tile([100, 128], f32r, tag=f"wb1_{kc}_{mc}_{tap}")
                        nc.sync.dma_start(
                            out=w_[:],
                            in_=wb1t_d[tap, kc * 100:(kc + 1) * 100,
                                       mc * 128:(mc + 1) * 128].bitcast(f32r))
                        wb1sb[kc, mc, tap] = w_
            wb2sb = {}
            for kc in range(2):
                for tap in range(3):
                    w_ = fr.tile([128, 128], f32r, tag=f"wb2_{kc}_{tap}")
                    nc.sync.dma_start(
                        out=w_[:],
                        in_=wb2t_d[tap, kc * 128:(kc + 1) * 128, :].bitcast(f32r))
                    wb2sb[kc, tap] = w_
            wt1sb = {}
            for mc in range(2):
                for tap in range(3):
                    w_ = fr.tile([128, 128], f32, tag=f"wt1_{mc}_{tap}")
                    nc.sync.dma_start(
                        out=w_[:],
                        in_=wt1t_d[tap, :, mc * 128:(mc + 1) * 128])
                    wt1sb[mc, tap] = w_
            wt2sb = {}
            for kc in range(2):
                for tap in range(3):
                    w_ = fr.tile([128, 2], f32, tag=f"wt2_{kc}_{tap}")
                    nc.sync.dma_start(
                        out=w_[:],
                        in_=wt2t_d[tap, kc * 128:(kc + 1) * 128, :])
                    wt2sb[kc, tap] = w_

            # base1 = relu(conv1d(x))  [256 -> 2 chunks][100 t x 4 b]
            base1_sb = []
            for mc in range(2):
                ps = frps.tile([128, BATCH, 100], f32, tag="ps_b1")
                first = True
                for kc in range(4):
                    for tap in range(3):
                        nc.tensor.matmul(ps[:], wb1sb[kc, mc, tap][:],
                                         x_sb[kc][:, :, tap:tap + 100],
                                         start=first, stop=(kc == 3 and tap == 2))
                        first = False
                t_ = fr.tile([128, BATCH, 102], f32r, tag=f"base1_{mc}")
                dma_zero(t_[:, :, 0:1])
                dma_zero(t_[:, :, 101:102])
                nc.scalar.activation(t_[:, :, 1:101], ps[:], AF.Relu, bias=b1sb[mc][:])
                base1_sb.append(t_)

            # base = relu(conv1d(base1))  [128][4 b x 102]
            ps = frps.tile([128, BATCH, 100], f32, tag="ps_b2")
            first = True
            for kc in range(2):
                for tap in range(3):
                    nc.tensor.matmul(ps[:], wb2sb[kc, tap][:],
                                     base1_sb[kc][:, :, tap:tap + 100],
                                     start=first, stop=(kc == 1 and tap == 2))
                    first = False
            base_sb = fr.tile([128, BATCH, 102], f32)
            nc.vector.memset(base_sb[:], 0.0)
            nc.scalar.activation(base_sb[:, :, 1:101], ps[:], AF.Relu, bias=b2sb[0][:])

            # tem1 = relu(conv1d(base)) [2 chunks][4 x 102]
            tem1_sb = []
            for mc in range(2):
                ps = frps.tile([128, BATCH, 100], f32, tag="ps_t1")
                first = True
                for tap in range(3):
                    nc.tensor.matmul(ps[:], wt1sb[mc, tap][:],
                                     base_sb[:, :, tap:tap + 100],
                                     start=first, stop=(tap == 2))
                    first = False
                t_ = fr.tile([128, BATCH, 102], f32, tag=f"tem1_{mc}")
                nc.vector.memset(t_[:], 0.0)
                nc.scalar.activation(t_[:, :, 1:101], ps[:], AF.Relu, bias=bt1sb[mc][:])
                tem1_sb.append(t_)

            # tem_out = sigmoid(conv1d(tem1)) [2][4 x 100]
            ps = frps.tile([128, BATCH, 100], f32, tag="ps_t2")
            first = True
            for kc in range(2):
                for tap in range(3):
                    nc.tensor.matmul(ps[:2], wt2sb[kc, tap][:],
                                     tem1_sb[kc][:, :, tap:tap + 100],
                                     start=first, stop=(kc == 1 and tap == 2))
                    first = False
            temsb = fr.tile([2, BATCH, 100], f32)
            nc.scalar.activation(temsb[:], ps[:2], AF.Sigmoid, bias=bt2sb[:])
            for b in range(BATCH):
                nc.sync.dma_start(out=tem_d[b], in_=temsb[:, b, :])

            # baseT[b] = base[:, b, 1:101].T  -> [100 tt, 128 c]
            for b in range(BATCH):
                ps = frps.tile([128, 128], f32, tag="ps_tr")
                nc.tensor.transpose(ps[:100, :], base_sb[:, b, 1:101], ident[:])
                nc.vector.tensor_copy(baseT[b][:], ps[:100, :])

        # w3r loads ride the sync queue behind the front's loads so they
        # neither delay the front nor stall the odd-n wsmp slices on gpsimd
        for n in range(NSMP):
            nc.sync.dma_start(out=w3rsb[n][:], in_=w3r_d[n])

        # ---------------- phase A: sampling + conv3d + 1x1 over packed cols
        with tc.tile_pool(name="pem_sb", bufs=4) as pempool, \
             tc.tile_pool(name="y2", bufs=2) as y2pool, \
             tc.tile_pool(name="pa_ps", bufs=3, space="PSUM") as paps, \
             tc.tile_pool(name="y_ps", bufs=1, space="PSUM") as yps, \
             tc.tile_pool(name="z_ps", bufs=1, space="PSUM") as zps:
            for blk in range(NBLK):
                cs = blk * CB
                for b in range(BATCH):
                    y_ps = [yps.tile([128, CB], f32, tag=f"y{oc}", name=f"y_ps{oc}")
                            for oc in range(4)]
                    # software-pipeline: pem runs 2 iterations ahead of the
                    # conv3d consumers so the PSUM->SBUF copy is off PE's
                    # critical path
                    pem_sbs = {}

                    def do_pem(n):
                        pem_ps = paps.tile([128, CB], f32, tag="pem",
                                           name="pem_ps")
                        nc.tensor.matmul(pem_ps[:], baseT[b][:],
                                         wsmpsb[blk, n][:],
                                         start=True, stop=True)
                        pem_sb = pempool.tile([128, CB], bf16, tag="pem_sb",
                                              name="pem_sb")
                        if n % 2 == 0:
                            nc.vector.tensor_copy(pem_sb[:], pem_ps[:])
                        else:
                            nc.scalar.activation(pem_sb[:], pem_ps[:], AF.Copy)
                        pem_sbs[n] = pem_sb

                    do_pem(0)
                    do_pem(1)
                    for n in range(NSMP):
                        if n + 2 < NSMP:
                            do_pem(n + 2)
                        pem_sb = pem_sbs.pop(n)
                        for oc in range(4):
                            nc.tensor.matmul(
                                y_ps[oc][:],
                                w3rsb[n][:, oc * 128:(oc + 1) * 128],
                                pem_sb[:],
                                start=(n == 0), stop=(n == NSMP - 1))
                    y2 = []
                    for oc in range(4):
                        t_ = y2pool.tile([128, CB], bf16, tag=f"y2_{oc}")
                        nc.scalar.activation(t_[:], y_ps[oc][:], AF.Relu,
                                             bias=b3sb[oc][:])
                        y2.append(t_)
                    z_ps = zps.tile([128, CB], f32, tag="z")
                    for oc in range(4):
                        nc.tensor.matmul(z_ps[:], w1x1sb[oc][:], y2[oc][:],
                                         start=(oc == 0), stop=(oc == 3))
                    # cols are t-major (5 t-rows of 100 d) -> z1[b][:, 5blk:5blk+5, 1:101]
                    tr0 = cs // 100
                    nc.scalar.activation(
                        z1[b][:, tr0:tr0 + CB // 100, 1:101],
                        z_ps[:].rearrange("p (a c) -> p a c", a=CB // 100),
                        AF.Relu, bias=b2d1sb[0][:])

        # zero d-pad cols and mask out-of-range t rows
        for b in range(BATCH):
            nc.vector.memset(z1[b][:, :, 0:1], 0.0)
            nc.vector.memset(z1[b][:, :, 101:102], 0.0)
            for tr in range(TW):
                nc.vector.tensor_scalar_mul(z1[b][:, tr, 1:101],
                                            z1[b][:, tr, 1:101],
                                            tvalsb[:, tr:tr + 1])

        # ---------------- phase B: 3x3 conv + relu, 1x1 + sigmoid, DMA out
        DCH = [(0, 36), (36, 36), (72, 28)]
        with tc.tile_pool(name="z2", bufs=2) as z2pool, \
             tc.tile_pool(name="pb_ps", bufs=2, space="PSUM") as pbps, \
             tc.tile_pool(name="yo_ps", bufs=2, space="PSUM") as yops, \
             tc.tile_pool(name="yout", bufs=3) as yopool:
            for b in range(BATCH):
                for (d0, dn) in DCH:
                    # out cols ordered (t, d): rhs slices keep d contiguous
                    zz = pbps.tile([128, TC, dn], f32, tag=f"zz{dn}",
                                   name=f"zz{dn}")
                    for kh in (-1, 0, 1):        # d shift
                        for kw in (-1, 0, 1):    # t shift
                            tap = (kh + 1) * 3 + (kw + 1)
                            rhs = z1[b][:, 1 + kw:1 + kw + TC,
                                        d0 + 1 + kh:d0 + 1 + kh + dn]
                            nc.tensor.matmul(zz[:], w2d2sb[tap][:], rhs,
                                             start=(tap == 0), stop=(tap == 8))
                    z2 = z2pool.tile([128, TC, dn], f32, tag=f"z2{dn}",
                                     name=f"z2{dn}")
                    nc.scalar.activation(z2[:], zz[:], AF.Relu,
                                         bias=b2d2sb[0][:])
                    yo = yops.tile([128, TC, dn], f32, tag=f"yo{dn}",
                                   name=f"yo{dn}")
                    nc.tensor.matmul(yo[:2], w2d3sb[:], z2[:],
                                     start=True, stop=True)
                    yout = yopool.tile([2, TC, dn], f32, tag=f"yout{dn}",
                                       name=f"yout{dn}")
                    nc.scalar.activation(yout[:], yo[:2], AF.Sigmoid,
                                         bias=b2d3sb[:])
                    nc.sync.dma_start(out=y_d[b, :, :, d0:d0 + dn],
                                      in_=yout[:])

        z1_pool.release()
        persist.release()

    nc.compile()
    return nc


# ---------------------------------------------------------------- entry point
def _make_in_maps(inputs):
    import ml_dtypes
    wsmps, tvals, pr = _prep_host(inputs)
    shared = {
        "wb1t": pr["wb1t"], "b_base1": inputs["b_base1"],
        "wb2t": pr["wb2t"], "b_base2": inputs["b_base2"],
        "wt1t": pr["wt1t"], "b_tem1": inputs["b_tem1"],
        "wt2t": pr["wt2t"], "b_tem2": inputs["b_tem2"],
        "w3r": pr["w3r"], "b_c3d": inputs["b_c3d"],
        "w1x1t": pr["w1x1t"], "b_2d1": inputs["b_2d1"],
        "w2d2t": pr["w2d2t"], "b_2d2": inputs["b_2d2"],
        "w2d3t": pr["w2d3t"], "b_2d3": inputs["b_2d3"],
        "x": inputs["x"],
        "zeros": np.zeros(512, np.float32),
        "zerosh": np.zeros(512, ml_dtypes.bfloat16),
    }
    in_maps = []
    for r in range(NCORES):
        m = dict(shared)
        m["wsmp"] = wsmps[r]
        m["tval"] = tvals[r]
        in_maps.append(m)
    return in_maps


def kernel(**inputs):
    inputs = {k: np.asarray(v, dtype=np.float32) for k, v in inputs.items()}

    if "nc" not in _cache:
        _cache["nc"] = _build_program()
    nc = _cache["nc"]

    in_maps = _make_in_maps(inputs)

    from concourse.bass_utils import run_bass_kernel_spmd
    res = run_bass_kernel_spmd(nc, in_maps, list(range(NCORES)))
    _cache["last_res"] = res

    y = np.zeros((BATCH, 2, DPROP, T), np.float32)
    for r in range(NCORES):
        t0 = r * TC
        t1 = min(T, t0 + TC)
        yr = res.results[r]["y"].transpose(0, 1, 3, 2)   # -> [b, 2, d, t]
        y[:, :, :, t0:t1] = yr[:, :, :, : t1 - t0]
    tem = res.results[0]["tem"]
    return tem, y
